# revision 37
# baseline (speedup 1.0000x reference)
"""Trainium2 Bass kernel for nn_ContextGatingSigmoidClassifier.

Math (eval mode):
  f_so = lrelu(W_so @ bn_so(x_so) + b_so)        x: [B,2048,N,H,W]
  f_c  = lrelu(W_c  @ bn_c(x_c)  + b_c)
  f    = concat -> bn1 -> W1 -> bn2 -> lrelu -> W2 -> mean(H,W) -> sigmoid > 0.5

All BatchNorms are eval-mode affine maps, so they fold into the adjacent
linear layers (done host-side in fp64). Final threshold:
  sigmoid(mean) > 0.5  <=>  sum_hw(W2 @ h) > -49*b2.

Device mapping: data-parallel over batch (4 per core, 8 cores), weights
replicated, x cast fp32->fp16 host-side. The kernel is tensor-engine
bound (1152 matmuls of N=294 at 125ns = 144us); the schedule exists to
keep the PE stream dense from the first possible cycle. Measured HW
facts that shape it:
  - ~7.1us of fixed BSP preamble before any kernel instruction.
  - Both HWDGE rings share ~420 GB/s; each transfer costs ~0.6-1us of
    fixed overhead (trigger + descgen), and the first ~4us of DMA
    activity runs at roughly half rate (cold clock).
  - A PE idle gap of more than ~2-3us re-gates the PE clock to 1.2GHz
    for 3.4us+ (HAM) - stalls snowball. So batch 0 runs layer 1 with
    an 8-PSUM-group schedule (4 o-chunks x 2 m-tiles at once): each
    k-chunk of data (x 147KB + weights 128KB) feeds 8 matmuls (1us),
    so PE demand (~275 GB/s) roughly matches even the cold DMA rate
    and per-chunk waits stay well under the HAM threshold.
  - Each of batch 0's branches gets its whole working set (4 weight
    o-blocks + x, 4.4MB) packed host-side into ONE k-ordered stream,
    split across both rings in alternating, geometrically growing
    transfers. The first 8 transfers use the 8 HWDGE semaphores at
    most once, so no critical trigger ever waits (the Tile framework
    recycles semaphores round-robin and a recycled trigger blocks on
    the previous user's completion); later triggers' recycled waits
    land on long-completed transfers.
  - w1 and the b1-b3 x loads carry manual scheduler delays
    (tile_wait_until) so the scheduler cannot hoist these dependency-
    free triggers in front of the critical streams.
  - PE pre-warm: dummy matmuls fill the PE queue from the preamble end
    (~7.4us) to the first k-chunk arrival (~10us).
  - Steady batches (b1-b3) use the o-major loop (2-deep PSUM slack vs
    activation latency), reading layer-1 weights from the resident
    stream tiles; all input DMA lands by ~70us of a ~165us kernel.
Per batch element: x[b] is [2048, 588] (channels on SBUF partitions in
K-chunks of 128, positions on the free dim, 2 m-tiles of 294 = one PSUM
bank each). Channel->partition mapping is interleaved (partition p holds
channels 16p..16p+15) so every DMA descriptor is one contiguous
per-partition run; weights are permuted host-side to match.
"""

import numpy as np

import concourse.bass as bass  # noqa: F401
import concourse.tile as tile
from concourse import bacc, mybir
from concourse.bass_utils import run_bass_kernel_spmd

F16 = mybir.dt.float16
F32 = mybir.dt.float32

B, C, NN, HW = 32, 2048, 12, 49
NHW = NN * HW            # 588
N_CORES = 8
BPC = B // N_CORES       # 4 batch elements per core
MT = NHW // 2            # 294 columns = one PSUM bank of fp32
KC1 = C // 128           # 16 K-chunks, layer 1
OC1 = 512 // 128         # 4 output chunks, layer 1 (per branch)
KC2 = 1024 // 128        # 8 K-chunks, layer 2
OC2 = 256 // 128         # 2 output chunks, layer 2
EPS = 1e-5
SLOPE = 0.2
N_DUMMY = 24             # PE pre-warm matmuls (~107ns each at 1.2GHz)
XO = OC1 * 128           # x column offset inside a combined chunk
RC = XO + NHW            # 1100 = combined chunk: weights (4 o) + x cols


def _quad(A):
    """[512, 2048] -> [128, KC1, 512] k-major weight layout:
    Wq[p, k, o, m] = A[128o+m, 16p+k] (chunk k holds channel 16p+k at
    partition p, matching the x DMA layout)."""
    A4 = A.reshape(OC1, 128, 128, KC1)               # [o, m, p, k]
    return np.ascontiguousarray(
        A4.transpose(2, 3, 0, 1).reshape(128, KC1, OC1 * 128))


def _fold_params(d):
    """Fold BNs into linears, in fp64. Returns device-layout arrays."""
    g = {k: np.asarray(v, dtype=np.float64) for k, v in d.items()}

    def bn_st(p):
        s = g[f"{p}_g"] / np.sqrt(g[f"{p}_v"] + EPS)
        t = g[f"{p}_b"] - g[f"{p}_m"] * s
        return s, t

    s_so, t_so = bn_st("bn_so")
    s_c, t_c = bn_st("bn_c")
    s1, t1 = bn_st("bn1")
    s2, t2 = bn_st("bn2")

    A_so = g["W_so"] * s_so[None, :]                 # [512, 2048]
    a_so = g["W_so"] @ t_so + g["b_so"]              # [512]
    A_c = g["W_c"] * s_c[None, :]
    a_c = g["W_c"] @ t_c + g["b_c"]
    A1 = s2[:, None] * (g["W1"] * s1[None, :])       # [256, 1024]
    a1 = s2 * (g["W1"] @ t1 + g["b1"]) + t2          # [256]

    wso = _quad(A_so).astype(np.float16)
    wc = _quad(A_c).astype(np.float16)
    # layer-2: W1_dev[p, o, k, m] = A1[128o+m, 128k+p] (f stores channel
    # 128k+p at partition p of column-block k).
    A4 = A1.reshape(OC2, 128, KC2, 128)              # [o, m, k, p]
    w1 = np.ascontiguousarray(
        A4.transpose(3, 0, 2, 1).reshape(128, OC2 * KC2 * 128)
    ).astype(np.float16)
    w2 = np.ascontiguousarray(g["W2"].reshape(OC2, 128).T).astype(np.float16)
    # bias pack [128, 10] fp32: bso(4) | bc(4) | b1(2)
    bias = np.concatenate([
        a_so.reshape(OC1, 128).T, a_c.reshape(OC1, 128).T,
        a1.reshape(OC2, 128).T], axis=1)
    bias = np.ascontiguousarray(bias).astype(np.float32)
    thresh = float(-HW * g["b2"][0])
    return wso, wc, w1, w2, bias, thresh


def build_bass(thresh, repeat=1, loop=1):
    nc = bacc.Bacc("TRN2", target_bir_lowering=False, debug=False)

    xso_d = nc.dram_tensor("x_so", [BPC, C, NHW], F16, kind="ExternalInput").ap()
    xc_d = nc.dram_tensor("x_c", [BPC, C, NHW], F16, kind="ExternalInput").ap()
    # combined streams: k-chunks of (all weight o-blocks | x[b0]) per
    # branch - each branch's whole working set in consumption order.
    cso_d = nc.dram_tensor("comb_so", [128, KC1 * RC], F16, kind="ExternalInput").ap()
    cc_d = nc.dram_tensor("comb_c", [128, KC1 * RC], F16, kind="ExternalInput").ap()
    w1_d = nc.dram_tensor("w1", [128, OC2 * KC2 * 128], F16, kind="ExternalInput").ap()
    w2_d = nc.dram_tensor("w2", [128, OC2], F16, kind="ExternalInput").ap()
    bias_d = nc.dram_tensor("bias", [128, 2 * OC1 + OC2], F32, kind="ExternalInput").ap()
    out_d = nc.dram_tensor("out", [BPC * NN], F32, kind="ExternalOutput").ap()

    with tile.TileContext(nc) as tc:
        with (
            tc.tile_pool(name="wp", bufs=1) as wp,
            tc.tile_pool(name="xp", bufs=2) as xp,
            tc.tile_pool(name="fp", bufs=2) as fp,
            tc.tile_pool(name="hp", bufs=2) as hp,
            tc.tile_pool(name="ap", bufs=1) as ac,
            tc.tile_pool(name="ps1", bufs=4, space="PSUM") as ps1,
            tc.tile_pool(name="ps2", bufs=2, space="PSUM") as ps2,
            tc.tile_pool(name="ps3", bufs=2, space="PSUM") as ps3,
        ):
            # ---- tiny tensors on the gpsimd (SWDGE) ring ----
            # memset first: it gates the PE pre-warm matmuls.
            dummy_sb = wp.tile([128, 128], F16)
            nc.gpsimd.memset(dummy_sb[:], 0)
            bias_sb = wp.tile([128, 2 * OC1 + OC2], F32)
            nc.gpsimd.dma_start(bias_sb[:], bias_d[:])
            w2_sb = wp.tile([128, OC2], F16)
            nc.gpsimd.dma_start(w2_sb[:], w2_d[:])

            # combined-stream tiles stay resident: steady batches read
            # layer-1 weights from them.
            cso_sb = wp.tile([128, KC1 * RC], F16)
            cc_sb = wp.tile([128, KC1 * RC], F16)
            w1_sb = wp.tile([128, OC2 * KC2 * 128], F16)

            # ---- PE pre-warm: HAM flips to 2.4GHz after ~3.4us of
            # activity; burn the DMA lead-in on dummy matmuls so the
            # real stream runs warm from the first real k-chunk.
            wps = ps1.tile([128, MT], F32, tag="ps1")
            for i in range(N_DUMMY):
                nc.tensor.matmul(wps[:, 0:128], lhsT=dummy_sb[:],
                                 rhs=dummy_sb[:], start=True, stop=True)

            out_sb = ac.tile([1, BPC * NN], F32)
            bits_sb = ac.tile([1, BPC * NN], F32)

            import contextlib
            loop_cm = tc.For_i(0, loop, 1) if loop > 1 else contextlib.nullcontext()
            with loop_cm:
                _body(nc, tc, repeat, xso_d, xc_d, (cso_d, cc_d, w1_d),
                      out_d, (cso_sb, cc_sb), w1_sb, w2_sb,
                      bias_sb, out_sb, bits_sb, xp, fp, hp,
                      ps1, ps2, ps3, thresh)

    nc.compile()
    return nc


def _body(nc, tc, repeat, xso_d, xc_d, comb_dram, out_d,
          comb_sb, w1_sb, w2_sb, bias_sb,
          out_sb, bits_sb, xp, fp, hp, ps1, ps2, ps3, thresh):
    cso_d, cc_d, w1_d = comb_dram
    comb_t = [sb.rearrange("p (k c) -> p k c", k=KC1) for sb in comb_sb]
    comb_v = [dd.rearrange("p (k c) -> p k c", k=KC1) for dd in (cso_d, cc_d)]

    def x_sub(ring, x_sb, x_d, b, lo, hi):
        """DMA k-chunks [lo, hi) of x[b]."""
        x_t = x_sb.rearrange("p (j m) -> p j m", j=KC1)
        x_v = x_d[b].rearrange("(p j) m -> p j m", p=128)
        ring.dma_start(x_t[:, lo:hi, :], x_v[:, lo:hi, :])

    def l1_lhsT(br, o):
        return lambda k: comb_t[br][:, k, o * 128:(o + 1) * 128]

    def l1_act(ps, f_sb, br, o, m):
        col = (br * OC1 + o) * NHW + m * MT
        boff = br * OC1 + o
        nc.scalar.activation(
            f_sb[:, col:col + MT], ps[:],
            mybir.ActivationFunctionType.Prelu,
            bias=bias_sb[:, boff:boff + 1], scale=1.0, alpha=SLOPE)

    def l3(b, h_sb, m3, last=False):
        """layer 3 (y = W2 @ h) for one m-half + 49-group mean-reduce."""
        ps = ps3.tile([1, MT], F32, tag="ps3")
        q_order = (1, 0) if (last and m3 == 1) else (0, 1)
        for qi, q in enumerate(q_order):
            nc.tensor.matmul(
                ps[:], lhsT=w2_sb[:, q:q + 1],
                rhs=h_sb[:, q * NHW + m3 * MT:q * NHW + m3 * MT + MT],
                start=(qi == 0), stop=(qi == OC2 - 1))
        off = b * NN + m3 * (MT // HW)
        nc.vector.reduce_sum(
            out_sb[0:1, off:off + MT // HW],
            ps.rearrange("p (g x) -> p g x", x=HW),
            axis=mybir.AxisListType.X)

    def finish(b):
        """threshold + store for batch b:
        sigmoid(mean) > 0.5  <=>  sum > -49*b2"""
        nc.vector.tensor_scalar(
            bits_sb[0:1, b * NN:(b + 1) * NN],
            out_sb[0:1, b * NN:(b + 1) * NN], float(thresh), None,
            mybir.AluOpType.is_gt)
        # b<last: gpsimd (SWDGE) ring - a sync-ring store would block
        # later x-transfer triggers queued behind it. Last batch: sync
        # ring (HWDGE completes ~0.6us faster, nothing queues after).
        ring_out = nc.sync if b == BPC - 1 else nc.gpsimd
        ring_out.dma_start(out_d[b * NN:(b + 1) * NN],
                           bits_sb[0:1, b * NN:(b + 1) * NN])

    pending = None

    def l1_8g(f_sb, br, rhs_of):
        """whole branch with 8 PSUM groups (o,m) open and k-chunks
        interleaved: each arriving chunk (x 147KB + w 128KB) feeds 8
        matmuls (1us), so PE demand matches even the cold DMA rate and
        per-chunk waits stay far below the HAM idle threshold."""
        lhs = [l1_lhsT(br, o) for o in range(OC1)]
        pools = (ps1, ps1, ps2, ps3)
        tags = ("ps1", "ps1", "ps2", "ps3")
        tiles = {}
        for o in range(OC1):
            for m in range(2):
                g_ps = pools[o].tile([128, MT], F32, tag=tags[o])
                tiles[(o, m)] = g_ps
        for k in range(KC1):
            for m in range(2):
                for o in range(OC1):
                    nc.tensor.matmul(
                        tiles[(o, m)][:], lhsT=lhs[o](k), rhs=rhs_of(k, m),
                        start=(k == 0), stop=(k == KC1 - 1))
        for o in range(OC1):
            for m in range(2):
                l1_act(tiles[(o, m)], f_sb, br, o, m)

    for _rep in range(repeat):
        for b in range(BPC):
            first = _rep == 0 and b == 0
            if first:
                # ---- ramp trigger schedule ----
                # the so-stream owns BOTH rings in alternating, growing
                # transfers (flat-column ranges; the first covers just
                # chunk 0's weights + m0 x so the first matmuls start
                # ~1.5us earlier); the c-stream follows, done by ~28us.
                # The first 8 triggers use the 8 HWDGE semaphores
                # exactly once, so no critical trigger ever waits.
                rings = (nc.sync, nc.scalar)
                so_cuts = (0, XO + MT, 2 * RC, 4 * RC, 6 * RC, 8 * RC,
                           10 * RC, 12 * RC, 14 * RC, KC1 * RC)
                for i, (lo, hi) in enumerate(
                        zip(so_cuts, so_cuts[1:])):
                    rings[i % 2].dma_start(comb_sb[0][:, lo:hi],
                                           cso_d[:, lo:hi])
                # single-chunk pieces through the mid-stream deficit
                # window (chunks 4-9 land right as cumulative delivery
                # runs ~2us behind PE demand; >2us PE gaps re-gate the
                # clock, so cap each wait at one chunk ~1.3us).
                c_cuts = (0, 2, 4, 5, 6, 7, 8, 9, 10, 12, 14, 16)
                for i, (lo, hi) in enumerate(zip(c_cuts, c_cuts[1:])):
                    rings[(i + 1) % 2].dma_start(
                        comb_t[1][:, lo:hi, :],
                        comb_v[1][:, lo:hi, :])
                with tc.tile_wait_until(0.018):
                    nc.scalar.dma_start(w1_sb[:], w1_d[:])
            else:
                # steady-state x loads: one sync-ring trigger per
                # tensor, scheduler-delayed so they queue behind the
                # batch-0 streams; data still lands ~30us before use.
                delay = 0.020 + 0.004 * (_rep * BPC + b)
                with tc.tile_wait_until(delay):
                    xso_sb = xp.tile([128, KC1 * NHW], F16, tag="xso")
                    x_sub(nc.sync, xso_sb, xso_d, b, 0, KC1)
                with tc.tile_wait_until(delay + 0.002):
                    xc_sb = xp.tile([128, KC1 * NHW], F16, tag="xc")
                    x_sub(nc.sync, xc_sb, xc_d, b, 0, KC1)

            # ---- layer 1: f = lrelu(A @ x + a), fp16 out ----
            f_sb = fp.tile([128, 2 * OC1 * NHW], F16, tag="f")
            if first:
                for br in range(2):
                    l1_8g(f_sb, br,
                          lambda k, m, br=br: comb_t[br][:, k,
                                                         XO + m * MT:
                                                         XO + (m + 1) * MT])
            else:
                for br, x_sb in enumerate((xso_sb, xc_sb)):
                    rhs = lambda k, m, x_sb=x_sb: x_sb[
                        :, k * NHW + m * MT:k * NHW + m * MT + MT]
                    for o in range(OC1):
                        for m in range(2):
                            ps = ps1.tile([128, MT], F32, tag="ps1")
                            for k in range(KC1):
                                nc.tensor.matmul(
                                    ps[:], lhsT=l1_lhsT(br, o)(k),
                                    rhs=rhs(k, m),
                                    start=(k == 0), stop=(k == KC1 - 1))
                            l1_act(ps, f_sb, br, o, m)
                            if br == 0 and o == 0 and m == 1 and pending:
                                # previous batch's deferred L3-m1 +
                                # threshold + store: its L2 acts are
                                # ~2us old now, so nothing stalls.
                                l3(*pending, 1)
                                finish(pending[0])
                                pending = None

            # ---- layer 2: h = lrelu(A1 @ f + a1); layer 3 + mean-reduce:
            # y = W2 @ h ; sum 49-groups. L3-m0 runs inline (its h acts
            # are old); L3-m1 + threshold + store are DEFERRED into the
            # next batch's L1 stream so the last L2 act's latency hides
            # under matmuls. Last batch: L3-m runs right after L2-m with
            # the o/q-order trick so only one act is on the tail.
            h_sb = hp.tile([128, OC2 * NHW], F16, tag="h")
            last = b == BPC - 1
            for m in range(2):
                # last batch, m1: o1 first so act(o1) runs under o0's
                # matmuls; L3 then accumulates q1 before q0 (exact - a
                # two-term fp32 add commutes), leaving only the o0 act
                # on the tail's critical path.
                o_order = (1, 0) if (last and m == 1) else (0, 1)
                for o in o_order:
                    ps = ps2.tile([128, MT], F32, tag="ps2")
                    for k in range(KC2):
                        nc.tensor.matmul(
                            ps[:],
                            lhsT=w1_sb[:, (o * KC2 + k) * 128:
                                       (o * KC2 + k) * 128 + 128],
                            rhs=f_sb[:, k * NHW + m * MT:
                                     k * NHW + m * MT + MT],
                            start=(k == 0), stop=(k == KC2 - 1))
                    col = o * NHW + m * MT
                    nc.scalar.activation(
                        h_sb[:, col:col + MT], ps[:],
                        mybir.ActivationFunctionType.Prelu,
                        bias=bias_sb[:, 2 * OC1 + o:2 * OC1 + o + 1],
                        scale=1.0, alpha=SLOPE)
                for m3 in ([m] if last else ([0] if m == 1 else [])):
                    l3(b, h_sb, m3, last)
            if last:
                finish(b)
            else:
                pending = (b, h_sb)


_CACHE = {}


def _get_nc(thresh, repeat=1, loop=1):
    key = (round(thresh, 9), repeat, loop)
    if key not in _CACHE:
        _CACHE[key] = build_bass(thresh, repeat, loop)
    return _CACHE[key]


def _prepare(inputs):
    """Fold params, cast x to fp16, build per-core input maps + nc."""
    wso, wc, w1, w2, bias, thresh = _fold_params(inputs)
    xso = np.asarray(inputs["x_so"], dtype=np.float32).reshape(
        B, C, NHW).astype(np.float16)
    xc = np.asarray(inputs["x_c"], dtype=np.float32).reshape(
        B, C, NHW).astype(np.float16)
    in_maps = []
    for i in range(N_CORES):
        # combined streams: per k-chunk, all weight o-blocks (512 cols)
        # then x[b0] (588 cols) - b0's data in consumption order.
        cso = np.concatenate(
            [wso, xso[i * BPC].reshape(128, KC1, NHW)], axis=2)
        cc = np.concatenate(
            [wc, xc[i * BPC].reshape(128, KC1, NHW)], axis=2)
        in_maps.append({
            "x_so": xso[i * BPC:(i + 1) * BPC],
            "x_c": xc[i * BPC:(i + 1) * BPC],
            "comb_so": np.ascontiguousarray(cso.reshape(128, KC1 * RC)),
            "comb_c": np.ascontiguousarray(cc.reshape(128, KC1 * RC)),
            "w1": w1, "w2": w2, "bias": bias,
        })
    return _get_nc(thresh), in_maps


def kernel(**inputs):
    nc, in_maps = _prepare(inputs)
    res = run_bass_kernel_spmd(nc, in_maps, list(range(N_CORES)))
    out = np.concatenate([res.results[i]["out"].reshape(BPC, NN)
                          for i in range(N_CORES)], axis=0)
    return np.ascontiguousarray(out.reshape(B, NN, 1).astype(np.float32))


# revision 39
# speedup vs baseline: 1.0177x; 1.0177x over previous
"""Trainium2 Bass kernel for nn_ContextGatingSigmoidClassifier.

Math (eval mode):
  f_so = lrelu(W_so @ bn_so(x_so) + b_so)        x: [B,2048,N,H,W]
  f_c  = lrelu(W_c  @ bn_c(x_c)  + b_c)
  f    = concat -> bn1 -> W1 -> bn2 -> lrelu -> W2 -> mean(H,W) -> sigmoid > 0.5

All BatchNorms are eval-mode affine maps, so they fold into the adjacent
linear layers (done host-side in fp64). Final threshold:
  sigmoid(mean) > 0.5  <=>  sum_hw(W2 @ h) > -49*b2.

Device mapping: data-parallel over batch (4 per core, 8 cores), weights
replicated, x cast fp32->fp16 host-side. The kernel is tensor-engine
bound (1152 matmuls of N=294 at 125ns = 144us); the schedule exists to
keep the PE stream dense from the first possible cycle. Measured HW
facts that shape it:
  - ~7.1us of fixed BSP preamble before any kernel instruction.
  - Both HWDGE rings share ~420 GB/s; each transfer costs ~0.6-1us of
    fixed overhead (trigger + descgen), and the first ~4us of DMA
    activity runs at roughly half rate (cold clock).
  - A PE idle gap of more than ~2-3us re-gates the PE clock to 1.2GHz
    for 3.4us+ (HAM) - stalls snowball. So batch 0 runs layer 1 with
    an 8-PSUM-group schedule (4 o-chunks x 2 m-tiles at once): each
    k-chunk of data (x 147KB + weights 128KB) feeds 8 matmuls (1us),
    so PE demand (~275 GB/s) roughly matches even the cold DMA rate
    and per-chunk waits stay well under the HAM threshold.
  - Each of batch 0's branches gets its whole working set (4 weight
    o-blocks + x, 4.4MB) packed host-side into ONE k-ordered stream,
    split across both rings in alternating, geometrically growing
    transfers. The first 8 transfers use the 8 HWDGE semaphores at
    most once, so no critical trigger ever waits (the Tile framework
    recycles semaphores round-robin and a recycled trigger blocks on
    the previous user's completion); later triggers' recycled waits
    land on long-completed transfers.
  - w1 and the b1-b3 x loads carry manual scheduler delays
    (tile_wait_until) so the scheduler cannot hoist these dependency-
    free triggers in front of the critical streams.
  - PE pre-warm: dummy matmuls fill the PE queue from the preamble end
    (~7.4us) to the first k-chunk arrival (~10us).
  - Steady batches (b1-b3) use the o-major loop (2-deep PSUM slack vs
    activation latency), reading layer-1 weights from the resident
    stream tiles; all input DMA lands by ~70us of a ~165us kernel.
Per batch element: x[b] is [2048, 588] (channels on SBUF partitions in
K-chunks of 128, positions on the free dim, 2 m-tiles of 294 = one PSUM
bank each). Channel->partition mapping is interleaved (partition p holds
channels 16p..16p+15) so every DMA descriptor is one contiguous
per-partition run; weights are permuted host-side to match.
"""

import numpy as np

import concourse.bass as bass  # noqa: F401
import concourse.tile as tile
from concourse import bacc, mybir
from concourse.bass_utils import run_bass_kernel_spmd

F16 = mybir.dt.float16
F32 = mybir.dt.float32

B, C, NN, HW = 32, 2048, 12, 49
NHW = NN * HW            # 588
N_CORES = 8
BPC = B // N_CORES       # 4 batch elements per core
MT = NHW // 2            # 294 columns = one PSUM bank of fp32
KC1 = C // 128           # 16 K-chunks, layer 1
OC1 = 512 // 128         # 4 output chunks, layer 1 (per branch)
KC2 = 1024 // 128        # 8 K-chunks, layer 2
OC2 = 256 // 128         # 2 output chunks, layer 2
EPS = 1e-5
SLOPE = 0.2
N_DUMMY = 22             # PE pre-warm matmuls (~107ns each at 1.2GHz)
XO = OC1 * 128           # x column offset inside a combined chunk
RC = XO + NHW            # 1100 = combined chunk: weights (4 o) + x cols


def _quad(A):
    """[512, 2048] -> [128, KC1, 512] k-major weight layout:
    Wq[p, k, o, m] = A[128o+m, 16p+k] (chunk k holds channel 16p+k at
    partition p, matching the x DMA layout)."""
    A4 = A.reshape(OC1, 128, 128, KC1)               # [o, m, p, k]
    return np.ascontiguousarray(
        A4.transpose(2, 3, 0, 1).reshape(128, KC1, OC1 * 128))


def _fold_params(d):
    """Fold BNs into linears, in fp64. Returns device-layout arrays."""
    g = {k: np.asarray(v, dtype=np.float64) for k, v in d.items()}

    def bn_st(p):
        s = g[f"{p}_g"] / np.sqrt(g[f"{p}_v"] + EPS)
        t = g[f"{p}_b"] - g[f"{p}_m"] * s
        return s, t

    s_so, t_so = bn_st("bn_so")
    s_c, t_c = bn_st("bn_c")
    s1, t1 = bn_st("bn1")
    s2, t2 = bn_st("bn2")

    A_so = g["W_so"] * s_so[None, :]                 # [512, 2048]
    a_so = g["W_so"] @ t_so + g["b_so"]              # [512]
    A_c = g["W_c"] * s_c[None, :]
    a_c = g["W_c"] @ t_c + g["b_c"]
    A1 = s2[:, None] * (g["W1"] * s1[None, :])       # [256, 1024]
    a1 = s2 * (g["W1"] @ t1 + g["b1"]) + t2          # [256]

    wso = _quad(A_so).astype(np.float16)
    wc = _quad(A_c).astype(np.float16)
    # layer-2: W1_dev[p, o, k, m] = A1[128o+m, 128k+p] (f stores channel
    # 128k+p at partition p of column-block k).
    A4 = A1.reshape(OC2, 128, KC2, 128)              # [o, m, k, p]
    w1 = np.ascontiguousarray(
        A4.transpose(3, 0, 2, 1).reshape(128, OC2 * KC2 * 128)
    ).astype(np.float16)
    w2 = np.ascontiguousarray(g["W2"].reshape(OC2, 128).T).astype(np.float16)
    # bias pack [128, 10] fp32: bso(4) | bc(4) | b1(2)
    bias = np.concatenate([
        a_so.reshape(OC1, 128).T, a_c.reshape(OC1, 128).T,
        a1.reshape(OC2, 128).T], axis=1)
    bias = np.ascontiguousarray(bias).astype(np.float32)
    thresh = float(-HW * g["b2"][0])
    return wso, wc, w1, w2, bias, thresh


def build_bass(thresh, repeat=1, loop=1):
    nc = bacc.Bacc("TRN2", target_bir_lowering=False, debug=False)

    xso_d = nc.dram_tensor("x_so", [BPC, C, NHW], F16, kind="ExternalInput").ap()
    xc_d = nc.dram_tensor("x_c", [BPC, C, NHW], F16, kind="ExternalInput").ap()
    # combined streams: k-chunks of (all weight o-blocks | x[b0]) per
    # branch - each branch's whole working set in consumption order.
    cso_d = nc.dram_tensor("comb_so", [128, KC1 * RC], F16, kind="ExternalInput").ap()
    cc_d = nc.dram_tensor("comb_c", [128, KC1 * RC], F16, kind="ExternalInput").ap()
    w1_d = nc.dram_tensor("w1", [128, OC2 * KC2 * 128], F16, kind="ExternalInput").ap()
    w2_d = nc.dram_tensor("w2", [128, OC2], F16, kind="ExternalInput").ap()
    bias_d = nc.dram_tensor("bias", [128, 2 * OC1 + OC2], F32, kind="ExternalInput").ap()
    out_d = nc.dram_tensor("out", [BPC * NN], F32, kind="ExternalOutput").ap()

    with tile.TileContext(nc) as tc:
        with (
            tc.tile_pool(name="wp", bufs=1) as wp,
            tc.tile_pool(name="xp", bufs=2) as xp,
            tc.tile_pool(name="fp", bufs=2) as fp,
            tc.tile_pool(name="hp", bufs=2) as hp,
            tc.tile_pool(name="ap", bufs=1) as ac,
            tc.tile_pool(name="ps1", bufs=4, space="PSUM") as ps1,
            tc.tile_pool(name="ps2", bufs=2, space="PSUM") as ps2,
            tc.tile_pool(name="ps3", bufs=2, space="PSUM") as ps3,
        ):
            # ---- tiny tensors on the gpsimd (SWDGE) ring ----
            # memset first: it gates the PE pre-warm matmuls.
            dummy_sb = wp.tile([128, 128], F16)
            nc.gpsimd.memset(dummy_sb[:], 0)
            bias_sb = wp.tile([128, 2 * OC1 + OC2], F32)
            nc.gpsimd.dma_start(bias_sb[:], bias_d[:])
            w2_sb = wp.tile([128, OC2], F16)
            nc.gpsimd.dma_start(w2_sb[:], w2_d[:])

            # combined-stream tiles stay resident: steady batches read
            # layer-1 weights from them.
            cso_sb = wp.tile([128, KC1 * RC], F16)
            cc_sb = wp.tile([128, KC1 * RC], F16)
            w1_sb = wp.tile([128, OC2 * KC2 * 128], F16)

            # ---- PE pre-warm: HAM flips to 2.4GHz after ~3.4us of
            # activity; burn the DMA lead-in on dummy matmuls so the
            # real stream runs warm from the first real k-chunk.
            wps = ps1.tile([128, MT], F32, tag="ps1")
            for i in range(N_DUMMY):
                nc.tensor.matmul(wps[:, 0:128], lhsT=dummy_sb[:],
                                 rhs=dummy_sb[:], start=True, stop=True)

            out_sb = ac.tile([1, BPC * NN], F32)
            bits_sb = ac.tile([1, BPC * NN], F32)

            import contextlib
            loop_cm = tc.For_i(0, loop, 1) if loop > 1 else contextlib.nullcontext()
            with loop_cm:
                _body(nc, tc, repeat, xso_d, xc_d, (cso_d, cc_d, w1_d),
                      out_d, (cso_sb, cc_sb), w1_sb, w2_sb,
                      bias_sb, out_sb, bits_sb, xp, fp, hp,
                      ps1, ps2, ps3, thresh)

    nc.compile()
    return nc


def _body(nc, tc, repeat, xso_d, xc_d, comb_dram, out_d,
          comb_sb, w1_sb, w2_sb, bias_sb,
          out_sb, bits_sb, xp, fp, hp, ps1, ps2, ps3, thresh):
    cso_d, cc_d, w1_d = comb_dram
    comb_t = [sb.rearrange("p (k c) -> p k c", k=KC1) for sb in comb_sb]
    comb_v = [dd.rearrange("p (k c) -> p k c", k=KC1) for dd in (cso_d, cc_d)]

    def x_sub(ring, x_sb, x_d, b, lo, hi):
        """DMA k-chunks [lo, hi) of x[b]."""
        x_t = x_sb.rearrange("p (j m) -> p j m", j=KC1)
        x_v = x_d[b].rearrange("(p j) m -> p j m", p=128)
        ring.dma_start(x_t[:, lo:hi, :], x_v[:, lo:hi, :])

    def l1_lhsT(br, o):
        return lambda k: comb_t[br][:, k, o * 128:(o + 1) * 128]

    def l1_act(ps, f_sb, br, o, m):
        col = (br * OC1 + o) * NHW + m * MT
        boff = br * OC1 + o
        nc.scalar.activation(
            f_sb[:, col:col + MT], ps[:],
            mybir.ActivationFunctionType.Prelu,
            bias=bias_sb[:, boff:boff + 1], scale=1.0, alpha=SLOPE)

    def l3(b, h_sb, m3, last=False):
        """layer 3 (y = W2 @ h) for one m-half + 49-group mean-reduce."""
        ps = ps3.tile([1, MT], F32, tag="ps3")
        q_order = (1, 0) if (last and m3 == 1) else (0, 1)
        for qi, q in enumerate(q_order):
            nc.tensor.matmul(
                ps[:], lhsT=w2_sb[:, q:q + 1],
                rhs=h_sb[:, q * NHW + m3 * MT:q * NHW + m3 * MT + MT],
                start=(qi == 0), stop=(qi == OC2 - 1))
        off = b * NN + m3 * (MT // HW)
        nc.vector.reduce_sum(
            out_sb[0:1, off:off + MT // HW],
            ps.rearrange("p (g x) -> p g x", x=HW),
            axis=mybir.AxisListType.X)

    def finish(b):
        """threshold + store for batch b:
        sigmoid(mean) > 0.5  <=>  sum > -49*b2"""
        nc.vector.tensor_scalar(
            bits_sb[0:1, b * NN:(b + 1) * NN],
            out_sb[0:1, b * NN:(b + 1) * NN], float(thresh), None,
            mybir.AluOpType.is_gt)
        # b<last: gpsimd (SWDGE) ring - a sync-ring store would block
        # later x-transfer triggers queued behind it. Last batch: sync
        # ring (HWDGE completes ~0.6us faster, nothing queues after).
        ring_out = nc.sync if b == BPC - 1 else nc.gpsimd
        ring_out.dma_start(out_d[b * NN:(b + 1) * NN],
                           bits_sb[0:1, b * NN:(b + 1) * NN])

    pending = None

    def l1_8g(f_sb, br, rhs_of):
        """whole branch with 8 PSUM groups (o,m) open and k-chunks
        interleaved: each arriving chunk (x 147KB + w 128KB) feeds 8
        matmuls (1us), so PE demand matches even the cold DMA rate and
        per-chunk waits stay far below the HAM idle threshold."""
        lhs = [l1_lhsT(br, o) for o in range(OC1)]
        pools = (ps1, ps1, ps2, ps3)
        tags = ("ps1", "ps1", "ps2", "ps3")
        tiles = {}
        for o in range(OC1):
            for m in range(2):
                g_ps = pools[o].tile([128, MT], F32, tag=tags[o])
                tiles[(o, m)] = g_ps
        for k in range(KC1):
            for m in range(2):
                for o in range(OC1):
                    nc.tensor.matmul(
                        tiles[(o, m)][:], lhsT=lhs[o](k), rhs=rhs_of(k, m),
                        start=(k == 0), stop=(k == KC1 - 1))
        for o in range(OC1):
            for m in range(2):
                l1_act(tiles[(o, m)], f_sb, br, o, m)

    for _rep in range(repeat):
        for b in range(BPC):
            first = _rep == 0 and b == 0
            if first:
                # ---- ramp trigger schedule ----
                # the so-stream owns BOTH rings in alternating, growing
                # transfers (flat-column ranges; the first covers just
                # chunk 0's weights + m0 x so the first matmuls start
                # ~1.5us earlier); the c-stream follows, done by ~28us.
                # The first 8 triggers use the 8 HWDGE semaphores
                # exactly once, so no critical trigger ever waits.
                rings = (nc.sync, nc.scalar)
                so_cuts = (0, XO + MT, 2 * RC, 4 * RC, 6 * RC, 8 * RC,
                           10 * RC, 12 * RC, 14 * RC, KC1 * RC)
                for i, (lo, hi) in enumerate(
                        zip(so_cuts, so_cuts[1:])):
                    rings[i % 2].dma_start(comb_sb[0][:, lo:hi],
                                           cso_d[:, lo:hi])
                for i in range(8):
                    rings[(i + 1) % 2].dma_start(
                        comb_t[1][:, 2 * i:2 * i + 2, :],
                        comb_v[1][:, 2 * i:2 * i + 2, :])
                with tc.tile_wait_until(0.018):
                    nc.scalar.dma_start(w1_sb[:], w1_d[:])
            else:
                # steady-state x loads: one sync-ring trigger per
                # tensor, scheduler-delayed so they queue behind the
                # batch-0 streams; data still lands ~30us before use.
                delay = 0.020 + 0.004 * (_rep * BPC + b)
                with tc.tile_wait_until(delay):
                    xso_sb = xp.tile([128, KC1 * NHW], F16, tag="xso")
                    x_sub(nc.sync, xso_sb, xso_d, b, 0, KC1)
                with tc.tile_wait_until(delay + 0.002):
                    xc_sb = xp.tile([128, KC1 * NHW], F16, tag="xc")
                    x_sub(nc.sync, xc_sb, xc_d, b, 0, KC1)

            # ---- layer 1: f = lrelu(A @ x + a), fp16 out ----
            f_sb = fp.tile([128, 2 * OC1 * NHW], F16, tag="f")
            if first:
                for br in range(2):
                    l1_8g(f_sb, br,
                          lambda k, m, br=br: comb_t[br][:, k,
                                                         XO + m * MT:
                                                         XO + (m + 1) * MT])
            else:
                for br, x_sb in enumerate((xso_sb, xc_sb)):
                    rhs = lambda k, m, x_sb=x_sb: x_sb[
                        :, k * NHW + m * MT:k * NHW + m * MT + MT]
                    for o in range(OC1):
                        for m in range(2):
                            ps = ps1.tile([128, MT], F32, tag="ps1")
                            for k in range(KC1):
                                nc.tensor.matmul(
                                    ps[:], lhsT=l1_lhsT(br, o)(k),
                                    rhs=rhs(k, m),
                                    start=(k == 0), stop=(k == KC1 - 1))
                            l1_act(ps, f_sb, br, o, m)
                            if br == 0 and o == 0 and m == 1 and pending:
                                # previous batch's deferred L3-m1 +
                                # threshold + store: its L2 acts are
                                # ~2us old now, so nothing stalls.
                                l3(*pending, 1)
                                finish(pending[0])
                                pending = None

            # ---- layer 2: h = lrelu(A1 @ f + a1); layer 3 + mean-reduce:
            # y = W2 @ h ; sum 49-groups. L3-m0 runs inline (its h acts
            # are old); L3-m1 + threshold + store are DEFERRED into the
            # next batch's L1 stream so the last L2 act's latency hides
            # under matmuls. Last batch: L3-m runs right after L2-m with
            # the o/q-order trick so only one act is on the tail.
            h_sb = hp.tile([128, OC2 * NHW], F16, tag="h")
            last = b == BPC - 1
            for m in range(2):
                # last batch, m1: o1 first so act(o1) runs under o0's
                # matmuls; L3 then accumulates q1 before q0 (exact - a
                # two-term fp32 add commutes), leaving only the o0 act
                # on the tail's critical path.
                o_order = (1, 0) if (last and m == 1) else (0, 1)
                for o in o_order:
                    ps = ps2.tile([128, MT], F32, tag="ps2")
                    for k in range(KC2):
                        nc.tensor.matmul(
                            ps[:],
                            lhsT=w1_sb[:, (o * KC2 + k) * 128:
                                       (o * KC2 + k) * 128 + 128],
                            rhs=f_sb[:, k * NHW + m * MT:
                                     k * NHW + m * MT + MT],
                            start=(k == 0), stop=(k == KC2 - 1))
                    col = o * NHW + m * MT
                    nc.scalar.activation(
                        h_sb[:, col:col + MT], ps[:],
                        mybir.ActivationFunctionType.Prelu,
                        bias=bias_sb[:, 2 * OC1 + o:2 * OC1 + o + 1],
                        scale=1.0, alpha=SLOPE)
                for m3 in ([m] if last else ([0] if m == 1 else [])):
                    l3(b, h_sb, m3, last)
            if last:
                finish(b)
            else:
                pending = (b, h_sb)


_CACHE = {}


def _get_nc(thresh, repeat=1, loop=1):
    key = (round(thresh, 9), repeat, loop)
    if key not in _CACHE:
        _CACHE[key] = build_bass(thresh, repeat, loop)
    return _CACHE[key]


def _prepare(inputs):
    """Fold params, cast x to fp16, build per-core input maps + nc."""
    wso, wc, w1, w2, bias, thresh = _fold_params(inputs)
    xso = np.asarray(inputs["x_so"], dtype=np.float32).reshape(
        B, C, NHW).astype(np.float16)
    xc = np.asarray(inputs["x_c"], dtype=np.float32).reshape(
        B, C, NHW).astype(np.float16)
    in_maps = []
    for i in range(N_CORES):
        # combined streams: per k-chunk, all weight o-blocks (512 cols)
        # then x[b0] (588 cols) - b0's data in consumption order.
        cso = np.concatenate(
            [wso, xso[i * BPC].reshape(128, KC1, NHW)], axis=2)
        cc = np.concatenate(
            [wc, xc[i * BPC].reshape(128, KC1, NHW)], axis=2)
        in_maps.append({
            "x_so": xso[i * BPC:(i + 1) * BPC],
            "x_c": xc[i * BPC:(i + 1) * BPC],
            "comb_so": np.ascontiguousarray(cso.reshape(128, KC1 * RC)),
            "comb_c": np.ascontiguousarray(cc.reshape(128, KC1 * RC)),
            "w1": w1, "w2": w2, "bias": bias,
        })
    return _get_nc(thresh), in_maps


def kernel(**inputs):
    nc, in_maps = _prepare(inputs)
    res = run_bass_kernel_spmd(nc, in_maps, list(range(N_CORES)))
    out = np.concatenate([res.results[i]["out"].reshape(BPC, NN)
                          for i in range(N_CORES)], axis=0)
    return np.ascontiguousarray(out.reshape(B, NN, 1).astype(np.float32))


# revision 40
# speedup vs baseline: 1.0448x; 1.0266x over previous
"""Trainium2 Bass kernel for nn_ContextGatingSigmoidClassifier.

Math (eval mode):
  f_so = lrelu(W_so @ bn_so(x_so) + b_so)        x: [B,2048,N,H,W]
  f_c  = lrelu(W_c  @ bn_c(x_c)  + b_c)
  f    = concat -> bn1 -> W1 -> bn2 -> lrelu -> W2 -> mean(H,W) -> sigmoid > 0.5

All BatchNorms are eval-mode affine maps, so they fold into the adjacent
linear layers (done host-side in fp64). Final threshold:
  sigmoid(mean) > 0.5  <=>  sum_hw(W2 @ h) > -49*b2.

Device mapping: data-parallel over batch (4 per core, 8 cores), weights
replicated, x cast fp32->fp16 host-side. The kernel is tensor-engine
bound (1152 matmuls of N=294 at 125ns = 144us); the schedule exists to
keep the PE stream dense from the first possible cycle. Measured HW
facts that shape it:
  - ~7.1us of fixed BSP preamble before any kernel instruction.
  - Both HWDGE rings share ~420 GB/s; each transfer costs ~0.6-1us of
    fixed overhead (trigger + descgen), and the first ~4us of DMA
    activity runs at roughly half rate (cold clock).
  - A PE idle gap of more than ~2-3us re-gates the PE clock to 1.2GHz
    for 3.4us+ (HAM) - stalls snowball. So batch 0 runs layer 1 with
    an 8-PSUM-group schedule (4 o-chunks x 2 m-tiles at once): each
    k-chunk of data (x 147KB + weights 128KB) feeds 8 matmuls (1us),
    so PE demand (~275 GB/s) roughly matches even the cold DMA rate
    and per-chunk waits stay well under the HAM threshold.
  - Each of batch 0's branches gets its whole working set (4 weight
    o-blocks + x, 4.4MB) packed host-side into ONE k-ordered stream,
    split across both rings in alternating, geometrically growing
    transfers. The first 8 transfers use the 8 HWDGE semaphores at
    most once, so no critical trigger ever waits (the Tile framework
    recycles semaphores round-robin and a recycled trigger blocks on
    the previous user's completion); later triggers' recycled waits
    land on long-completed transfers.
  - w1 and the b1-b3 x loads carry manual scheduler delays
    (tile_wait_until) so the scheduler cannot hoist these dependency-
    free triggers in front of the critical streams.
  - PE pre-warm: dummy matmuls fill the PE queue from the preamble end
    (~7.4us) to the first k-chunk arrival (~10us).
  - Steady batches (b1-b3) use the o-major loop (2-deep PSUM slack vs
    activation latency), reading layer-1 weights from the resident
    stream tiles; all input DMA lands by ~70us of a ~165us kernel.
Per batch element: x[b] is [2048, 588] (channels on SBUF partitions in
K-chunks of 128, positions on the free dim, 2 m-tiles of 294 = one PSUM
bank each). Channel->partition mapping is interleaved (partition p holds
channels 16p..16p+15) so every DMA descriptor is one contiguous
per-partition run; weights are permuted host-side to match.
"""

import numpy as np

import concourse.bass as bass  # noqa: F401
import concourse.tile as tile
from concourse import bacc, mybir
from concourse.bass_utils import run_bass_kernel_spmd

F16 = mybir.dt.float16
F32 = mybir.dt.float32

B, C, NN, HW = 32, 2048, 12, 49
NHW = NN * HW            # 588
N_CORES = 8
BPC = B // N_CORES       # 4 batch elements per core
MT = NHW // 2            # 294 columns = one PSUM bank of fp32
KC1 = C // 128           # 16 K-chunks, layer 1
OC1 = 512 // 128         # 4 output chunks, layer 1 (per branch)
KC2 = 1024 // 128        # 8 K-chunks, layer 2
OC2 = 256 // 128         # 2 output chunks, layer 2
EPS = 1e-5
SLOPE = 0.2
N_DUMMY = 32             # PE pre-warm matmuls (~107ns each at 1.2GHz);
                         # sized to bridge the PE from the preamble end
                         # (~7.4us) to k0 arrival (~10.8us) with NO gap:
                         # the HAM clock needs ~3.4us of continuous
                         # activity, and an idle gap resets it (costing
                         # ~4us of half-clock on the real stream).
XO = OC1 * 128           # x column offset inside a combined chunk
RC = XO + NHW            # 1100 = combined chunk: weights (4 o) + x cols


def _quad(A):
    """[512, 2048] -> [128, KC1, 512] k-major weight layout:
    Wq[p, k, o, m] = A[128o+m, 16p+k] (chunk k holds channel 16p+k at
    partition p, matching the x DMA layout)."""
    A4 = A.reshape(OC1, 128, 128, KC1)               # [o, m, p, k]
    return np.ascontiguousarray(
        A4.transpose(2, 3, 0, 1).reshape(128, KC1, OC1 * 128))


def _fold_params(d):
    """Fold BNs into linears, in fp64. Returns device-layout arrays."""
    g = {k: np.asarray(v, dtype=np.float64) for k, v in d.items()}

    def bn_st(p):
        s = g[f"{p}_g"] / np.sqrt(g[f"{p}_v"] + EPS)
        t = g[f"{p}_b"] - g[f"{p}_m"] * s
        return s, t

    s_so, t_so = bn_st("bn_so")
    s_c, t_c = bn_st("bn_c")
    s1, t1 = bn_st("bn1")
    s2, t2 = bn_st("bn2")

    A_so = g["W_so"] * s_so[None, :]                 # [512, 2048]
    a_so = g["W_so"] @ t_so + g["b_so"]              # [512]
    A_c = g["W_c"] * s_c[None, :]
    a_c = g["W_c"] @ t_c + g["b_c"]
    A1 = s2[:, None] * (g["W1"] * s1[None, :])       # [256, 1024]
    a1 = s2 * (g["W1"] @ t1 + g["b1"]) + t2          # [256]

    wso = _quad(A_so).astype(np.float16)
    wc = _quad(A_c).astype(np.float16)
    # layer-2: W1_dev[p, o, k, m] = A1[128o+m, 128k+p] (f stores channel
    # 128k+p at partition p of column-block k).
    A4 = A1.reshape(OC2, 128, KC2, 128)              # [o, m, k, p]
    w1 = np.ascontiguousarray(
        A4.transpose(3, 0, 2, 1).reshape(128, OC2 * KC2 * 128)
    ).astype(np.float16)
    w2 = np.ascontiguousarray(g["W2"].reshape(OC2, 128).T).astype(np.float16)
    # bias pack [128, 10] fp32: bso(4) | bc(4) | b1(2)
    bias = np.concatenate([
        a_so.reshape(OC1, 128).T, a_c.reshape(OC1, 128).T,
        a1.reshape(OC2, 128).T], axis=1)
    bias = np.ascontiguousarray(bias).astype(np.float32)
    thresh = float(-HW * g["b2"][0])
    return wso, wc, w1, w2, bias, thresh


def build_bass(thresh, repeat=1, loop=1):
    nc = bacc.Bacc("TRN2", target_bir_lowering=False, debug=False)

    xso_d = nc.dram_tensor("x_so", [BPC, C, NHW], F16, kind="ExternalInput").ap()
    xc_d = nc.dram_tensor("x_c", [BPC, C, NHW], F16, kind="ExternalInput").ap()
    # combined streams: k-chunks of (all weight o-blocks | x[b0]) per
    # branch - each branch's whole working set in consumption order.
    cso_d = nc.dram_tensor("comb_so", [128, KC1 * RC], F16, kind="ExternalInput").ap()
    cc_d = nc.dram_tensor("comb_c", [128, KC1 * RC], F16, kind="ExternalInput").ap()
    w1_d = nc.dram_tensor("w1", [128, OC2 * KC2 * 128], F16, kind="ExternalInput").ap()
    w2_d = nc.dram_tensor("w2", [128, OC2], F16, kind="ExternalInput").ap()
    bias_d = nc.dram_tensor("bias", [128, 2 * OC1 + OC2], F32, kind="ExternalInput").ap()
    out_d = nc.dram_tensor("out", [BPC * NN], F32, kind="ExternalOutput").ap()

    with tile.TileContext(nc) as tc:
        with (
            tc.tile_pool(name="wp", bufs=1) as wp,
            tc.tile_pool(name="xp", bufs=2) as xp,
            tc.tile_pool(name="fp", bufs=2) as fp,
            tc.tile_pool(name="hp", bufs=2) as hp,
            tc.tile_pool(name="ap", bufs=1) as ac,
            tc.tile_pool(name="ps1", bufs=4, space="PSUM") as ps1,
            tc.tile_pool(name="ps2", bufs=2, space="PSUM") as ps2,
            tc.tile_pool(name="ps3", bufs=2, space="PSUM") as ps3,
        ):
            # ---- tiny tensors on the gpsimd (SWDGE) ring ----
            # memset first: it gates the PE pre-warm matmuls.
            dummy_sb = wp.tile([128, 128], F16)
            nc.gpsimd.memset(dummy_sb[:], 0)
            bias_sb = wp.tile([128, 2 * OC1 + OC2], F32)
            nc.gpsimd.dma_start(bias_sb[:], bias_d[:])
            w2_sb = wp.tile([128, OC2], F16)
            nc.gpsimd.dma_start(w2_sb[:], w2_d[:])

            # combined-stream tiles stay resident: steady batches read
            # layer-1 weights from them.
            cso_sb = wp.tile([128, KC1 * RC], F16)
            cc_sb = wp.tile([128, KC1 * RC], F16)
            w1_sb = wp.tile([128, OC2 * KC2 * 128], F16)

            # ---- PE pre-warm: HAM flips to 2.4GHz after ~3.4us of
            # activity; burn the DMA lead-in on dummy matmuls so the
            # real stream runs warm from the first real k-chunk.
            wps = ps1.tile([128, MT], F32, tag="ps1")
            for i in range(N_DUMMY):
                nc.tensor.matmul(wps[:, 0:128], lhsT=dummy_sb[:],
                                 rhs=dummy_sb[:], start=True, stop=True)

            out_sb = ac.tile([1, BPC * NN], F32)
            bits_sb = ac.tile([1, BPC * NN], F32)

            import contextlib
            loop_cm = tc.For_i(0, loop, 1) if loop > 1 else contextlib.nullcontext()
            with loop_cm:
                _body(nc, tc, repeat, xso_d, xc_d, (cso_d, cc_d, w1_d),
                      out_d, (cso_sb, cc_sb), w1_sb, w2_sb,
                      bias_sb, out_sb, bits_sb, xp, fp, hp,
                      ps1, ps2, ps3, thresh)

    nc.compile()
    return nc


def _body(nc, tc, repeat, xso_d, xc_d, comb_dram, out_d,
          comb_sb, w1_sb, w2_sb, bias_sb,
          out_sb, bits_sb, xp, fp, hp, ps1, ps2, ps3, thresh):
    cso_d, cc_d, w1_d = comb_dram
    comb_t = [sb.rearrange("p (k c) -> p k c", k=KC1) for sb in comb_sb]
    comb_v = [dd.rearrange("p (k c) -> p k c", k=KC1) for dd in (cso_d, cc_d)]

    def x_sub(ring, x_sb, x_d, b, lo, hi):
        """DMA k-chunks [lo, hi) of x[b]."""
        x_t = x_sb.rearrange("p (j m) -> p j m", j=KC1)
        x_v = x_d[b].rearrange("(p j) m -> p j m", p=128)
        ring.dma_start(x_t[:, lo:hi, :], x_v[:, lo:hi, :])

    def l1_lhsT(br, o):
        return lambda k: comb_t[br][:, k, o * 128:(o + 1) * 128]

    def l1_act(ps, f_sb, br, o, m):
        col = (br * OC1 + o) * NHW + m * MT
        boff = br * OC1 + o
        nc.scalar.activation(
            f_sb[:, col:col + MT], ps[:],
            mybir.ActivationFunctionType.Prelu,
            bias=bias_sb[:, boff:boff + 1], scale=1.0, alpha=SLOPE)

    def l3(b, h_sb, m3, last=False):
        """layer 3 (y = W2 @ h) for one m-half + 49-group mean-reduce."""
        ps = ps3.tile([1, MT], F32, tag="ps3")
        q_order = (1, 0) if (last and m3 == 1) else (0, 1)
        for qi, q in enumerate(q_order):
            nc.tensor.matmul(
                ps[:], lhsT=w2_sb[:, q:q + 1],
                rhs=h_sb[:, q * NHW + m3 * MT:q * NHW + m3 * MT + MT],
                start=(qi == 0), stop=(qi == OC2 - 1))
        off = b * NN + m3 * (MT // HW)
        nc.vector.reduce_sum(
            out_sb[0:1, off:off + MT // HW],
            ps.rearrange("p (g x) -> p g x", x=HW),
            axis=mybir.AxisListType.X)

    def finish(b):
        """threshold + store for batch b:
        sigmoid(mean) > 0.5  <=>  sum > -49*b2"""
        nc.vector.tensor_scalar(
            bits_sb[0:1, b * NN:(b + 1) * NN],
            out_sb[0:1, b * NN:(b + 1) * NN], float(thresh), None,
            mybir.AluOpType.is_gt)
        # b<last: gpsimd (SWDGE) ring - a sync-ring store would block
        # later x-transfer triggers queued behind it. Last batch: sync
        # ring (HWDGE completes ~0.6us faster, nothing queues after).
        ring_out = nc.sync if b == BPC - 1 else nc.gpsimd
        ring_out.dma_start(out_d[b * NN:(b + 1) * NN],
                           bits_sb[0:1, b * NN:(b + 1) * NN])

    pending = None

    def l1_8g(f_sb, br, rhs_of):
        """whole branch with 8 PSUM groups (o,m) open and k-chunks
        interleaved: each arriving chunk (x 147KB + w 128KB) feeds 8
        matmuls (1us), so PE demand matches even the cold DMA rate and
        per-chunk waits stay far below the HAM idle threshold."""
        lhs = [l1_lhsT(br, o) for o in range(OC1)]
        pools = (ps1, ps1, ps2, ps3)
        tags = ("ps1", "ps1", "ps2", "ps3")
        tiles = {}
        for o in range(OC1):
            for m in range(2):
                g_ps = pools[o].tile([128, MT], F32, tag=tags[o])
                tiles[(o, m)] = g_ps
        for k in range(KC1):
            for m in range(2):
                for o in range(OC1):
                    nc.tensor.matmul(
                        tiles[(o, m)][:], lhsT=lhs[o](k), rhs=rhs_of(k, m),
                        start=(k == 0), stop=(k == KC1 - 1))
        for o in range(OC1):
            for m in range(2):
                l1_act(tiles[(o, m)], f_sb, br, o, m)

    for _rep in range(repeat):
        for b in range(BPC):
            first = _rep == 0 and b == 0
            if first:
                # ---- ramp trigger schedule ----
                # the so-stream owns BOTH rings in alternating, growing
                # transfers (flat-column ranges; the first covers just
                # chunk 0's weights + m0 x so the first matmuls start
                # ~1.5us earlier); the c-stream follows, done by ~28us.
                # The first 8 triggers use the 8 HWDGE semaphores
                # exactly once, so no critical trigger ever waits.
                rings = (nc.sync, nc.scalar)
                so_cuts = (0, XO + MT, 2 * RC, 4 * RC, 6 * RC, 8 * RC,
                           10 * RC, 12 * RC, 14 * RC, KC1 * RC)
                for i, (lo, hi) in enumerate(
                        zip(so_cuts, so_cuts[1:])):
                    rings[i % 2].dma_start(comb_sb[0][:, lo:hi],
                                           cso_d[:, lo:hi])
                for i in range(8):
                    rings[(i + 1) % 2].dma_start(
                        comb_t[1][:, 2 * i:2 * i + 2, :],
                        comb_v[1][:, 2 * i:2 * i + 2, :])
                with tc.tile_wait_until(0.018):
                    nc.scalar.dma_start(w1_sb[:], w1_d[:])
            else:
                # steady-state x loads: one sync-ring trigger per
                # tensor, scheduler-delayed so they queue behind the
                # batch-0 streams; data still lands ~30us before use.
                delay = 0.020 + 0.004 * (_rep * BPC + b)
                with tc.tile_wait_until(delay):
                    xso_sb = xp.tile([128, KC1 * NHW], F16, tag="xso")
                    x_sub(nc.sync, xso_sb, xso_d, b, 0, KC1)
                with tc.tile_wait_until(delay + 0.002):
                    xc_sb = xp.tile([128, KC1 * NHW], F16, tag="xc")
                    x_sub(nc.sync, xc_sb, xc_d, b, 0, KC1)

            # ---- layer 1: f = lrelu(A @ x + a), fp16 out ----
            f_sb = fp.tile([128, 2 * OC1 * NHW], F16, tag="f")
            if first:
                for br in range(2):
                    l1_8g(f_sb, br,
                          lambda k, m, br=br: comb_t[br][:, k,
                                                         XO + m * MT:
                                                         XO + (m + 1) * MT])
            else:
                for br, x_sb in enumerate((xso_sb, xc_sb)):
                    rhs = lambda k, m, x_sb=x_sb: x_sb[
                        :, k * NHW + m * MT:k * NHW + m * MT + MT]
                    for o in range(OC1):
                        for m in range(2):
                            ps = ps1.tile([128, MT], F32, tag="ps1")
                            for k in range(KC1):
                                nc.tensor.matmul(
                                    ps[:], lhsT=l1_lhsT(br, o)(k),
                                    rhs=rhs(k, m),
                                    start=(k == 0), stop=(k == KC1 - 1))
                            l1_act(ps, f_sb, br, o, m)
                            if br == 0 and o == 0 and m == 1 and pending:
                                # previous batch's deferred L3-m1 +
                                # threshold + store: its L2 acts are
                                # ~2us old now, so nothing stalls.
                                l3(*pending, 1)
                                finish(pending[0])
                                pending = None

            # ---- layer 2: h = lrelu(A1 @ f + a1); layer 3 + mean-reduce:
            # y = W2 @ h ; sum 49-groups. L3-m0 runs inline (its h acts
            # are old); L3-m1 + threshold + store are DEFERRED into the
            # next batch's L1 stream so the last L2 act's latency hides
            # under matmuls. Last batch: L3-m runs right after L2-m with
            # the o/q-order trick so only one act is on the tail.
            h_sb = hp.tile([128, OC2 * NHW], F16, tag="h")
            last = b == BPC - 1
            for m in range(2):
                # last batch, m1: o1 first so act(o1) runs under o0's
                # matmuls; L3 then accumulates q1 before q0 (exact - a
                # two-term fp32 add commutes), leaving only the o0 act
                # on the tail's critical path.
                o_order = (1, 0) if (last and m == 1) else (0, 1)
                for o in o_order:
                    ps = ps2.tile([128, MT], F32, tag="ps2")
                    for k in range(KC2):
                        nc.tensor.matmul(
                            ps[:],
                            lhsT=w1_sb[:, (o * KC2 + k) * 128:
                                       (o * KC2 + k) * 128 + 128],
                            rhs=f_sb[:, k * NHW + m * MT:
                                     k * NHW + m * MT + MT],
                            start=(k == 0), stop=(k == KC2 - 1))
                    col = o * NHW + m * MT
                    nc.scalar.activation(
                        h_sb[:, col:col + MT], ps[:],
                        mybir.ActivationFunctionType.Prelu,
                        bias=bias_sb[:, 2 * OC1 + o:2 * OC1 + o + 1],
                        scale=1.0, alpha=SLOPE)
                for m3 in ([m] if last else ([0] if m == 1 else [])):
                    l3(b, h_sb, m3, last)
            if last:
                finish(b)
            else:
                pending = (b, h_sb)


_CACHE = {}


def _get_nc(thresh, repeat=1, loop=1):
    key = (round(thresh, 9), repeat, loop)
    if key not in _CACHE:
        _CACHE[key] = build_bass(thresh, repeat, loop)
    return _CACHE[key]


def _prepare(inputs):
    """Fold params, cast x to fp16, build per-core input maps + nc."""
    wso, wc, w1, w2, bias, thresh = _fold_params(inputs)
    xso = np.asarray(inputs["x_so"], dtype=np.float32).reshape(
        B, C, NHW).astype(np.float16)
    xc = np.asarray(inputs["x_c"], dtype=np.float32).reshape(
        B, C, NHW).astype(np.float16)
    in_maps = []
    for i in range(N_CORES):
        # combined streams: per k-chunk, all weight o-blocks (512 cols)
        # then x[b0] (588 cols) - b0's data in consumption order.
        cso = np.concatenate(
            [wso, xso[i * BPC].reshape(128, KC1, NHW)], axis=2)
        cc = np.concatenate(
            [wc, xc[i * BPC].reshape(128, KC1, NHW)], axis=2)
        in_maps.append({
            "x_so": xso[i * BPC:(i + 1) * BPC],
            "x_c": xc[i * BPC:(i + 1) * BPC],
            "comb_so": np.ascontiguousarray(cso.reshape(128, KC1 * RC)),
            "comb_c": np.ascontiguousarray(cc.reshape(128, KC1 * RC)),
            "w1": w1, "w2": w2, "bias": bias,
        })
    return _get_nc(thresh), in_maps


def kernel(**inputs):
    nc, in_maps = _prepare(inputs)
    res = run_bass_kernel_spmd(nc, in_maps, list(range(N_CORES)))
    out = np.concatenate([res.results[i]["out"].reshape(BPC, NN)
                          for i in range(N_CORES)], axis=0)
    return np.ascontiguousarray(out.reshape(B, NN, 1).astype(np.float32))


# revision 41
# speedup vs baseline: 1.0563x; 1.0110x over previous
"""Trainium2 Bass kernel for nn_ContextGatingSigmoidClassifier.

Math (eval mode):
  f_so = lrelu(W_so @ bn_so(x_so) + b_so)        x: [B,2048,N,H,W]
  f_c  = lrelu(W_c  @ bn_c(x_c)  + b_c)
  f    = concat -> bn1 -> W1 -> bn2 -> lrelu -> W2 -> mean(H,W) -> sigmoid > 0.5

All BatchNorms are eval-mode affine maps, so they fold into the adjacent
linear layers (done host-side in fp64). Final threshold:
  sigmoid(mean) > 0.5  <=>  sum_hw(W2 @ h) > -49*b2.

Device mapping: data-parallel over batch (4 per core, 8 cores), weights
replicated, x cast fp32->fp16 host-side. The kernel is tensor-engine
bound (1152 matmuls of N=294 at 125ns = 144us); the schedule exists to
keep the PE stream dense from the first possible cycle. Measured HW
facts that shape it:
  - ~7.1us of fixed BSP preamble before any kernel instruction.
  - Both HWDGE rings share ~420 GB/s; each transfer costs ~0.6-1us of
    fixed overhead (trigger + descgen), and the first ~4us of DMA
    activity runs at roughly half rate (cold clock).
  - A PE idle gap of more than ~2-3us re-gates the PE clock to 1.2GHz
    for 3.4us+ (HAM) - stalls snowball. So batch 0 runs layer 1 with
    an 8-PSUM-group schedule (4 o-chunks x 2 m-tiles at once): each
    k-chunk of data (x 147KB + weights 128KB) feeds 8 matmuls (1us),
    so PE demand (~275 GB/s) roughly matches even the cold DMA rate
    and per-chunk waits stay well under the HAM threshold.
  - Each of batch 0's branches gets its whole working set (4 weight
    o-blocks + x, 4.4MB) packed host-side into ONE k-ordered stream,
    split across both rings in alternating, geometrically growing
    transfers. The first 8 transfers use the 8 HWDGE semaphores at
    most once, so no critical trigger ever waits (the Tile framework
    recycles semaphores round-robin and a recycled trigger blocks on
    the previous user's completion); later triggers' recycled waits
    land on long-completed transfers.
  - w1 and the b1-b3 x loads carry manual scheduler delays
    (tile_wait_until) so the scheduler cannot hoist these dependency-
    free triggers in front of the critical streams.
  - PE pre-warm: dummy matmuls fill the PE queue from the preamble end
    (~7.4us) to the first k-chunk arrival (~10us).
  - Steady batches (b1-b3) use the o-major loop (2-deep PSUM slack vs
    activation latency), reading layer-1 weights from the resident
    stream tiles; all input DMA lands by ~70us of a ~165us kernel.
Per batch element: x[b] is [2048, 588] (channels on SBUF partitions in
K-chunks of 128, positions on the free dim, 2 m-tiles of 294 = one PSUM
bank each). Channel->partition mapping is interleaved (partition p holds
channels 16p..16p+15) so every DMA descriptor is one contiguous
per-partition run; weights are permuted host-side to match.
"""

import numpy as np

import concourse.bass as bass  # noqa: F401
import concourse.tile as tile
from concourse import bacc, mybir
from concourse.bass_utils import run_bass_kernel_spmd

F16 = mybir.dt.float16
F32 = mybir.dt.float32

B, C, NN, HW = 32, 2048, 12, 49
NHW = NN * HW            # 588
N_CORES = 8
BPC = B // N_CORES       # 4 batch elements per core
MT = NHW // 2            # 294 columns = one PSUM bank of fp32
KC1 = C // 128           # 16 K-chunks, layer 1
OC1 = 512 // 128         # 4 output chunks, layer 1 (per branch)
KC2 = 1024 // 128        # 8 K-chunks, layer 2
OC2 = 256 // 128         # 2 output chunks, layer 2
EPS = 1e-5
SLOPE = 0.2
N_DUMMY = 32             # PE pre-warm matmuls (~107ns each at 1.2GHz);
                         # sized to bridge the PE from the preamble end
                         # (~7.4us) to k0 arrival (~10.8us) with NO gap:
                         # the HAM clock needs ~3.4us of continuous
                         # activity, and an idle gap resets it (costing
                         # ~4us of half-clock on the real stream).
XO = OC1 * 128           # x column offset inside a combined chunk
RC = XO + NHW            # 1100 = combined chunk: weights (4 o) + x cols


def _quad(A):
    """[512, 2048] -> [128, KC1, 512] k-major weight layout:
    Wq[p, k, o, m] = A[128o+m, 16p+k] (chunk k holds channel 16p+k at
    partition p, matching the x DMA layout)."""
    A4 = A.reshape(OC1, 128, 128, KC1)               # [o, m, p, k]
    return np.ascontiguousarray(
        A4.transpose(2, 3, 0, 1).reshape(128, KC1, OC1 * 128))


def _fold_params(d):
    """Fold BNs into linears, in fp64. Returns device-layout arrays."""
    g = {k: np.asarray(v, dtype=np.float64) for k, v in d.items()}

    def bn_st(p):
        s = g[f"{p}_g"] / np.sqrt(g[f"{p}_v"] + EPS)
        t = g[f"{p}_b"] - g[f"{p}_m"] * s
        return s, t

    s_so, t_so = bn_st("bn_so")
    s_c, t_c = bn_st("bn_c")
    s1, t1 = bn_st("bn1")
    s2, t2 = bn_st("bn2")

    A_so = g["W_so"] * s_so[None, :]                 # [512, 2048]
    a_so = g["W_so"] @ t_so + g["b_so"]              # [512]
    A_c = g["W_c"] * s_c[None, :]
    a_c = g["W_c"] @ t_c + g["b_c"]
    A1 = s2[:, None] * (g["W1"] * s1[None, :])       # [256, 1024]
    a1 = s2 * (g["W1"] @ t1 + g["b1"]) + t2          # [256]

    wso = _quad(A_so).astype(np.float16)
    wc = _quad(A_c).astype(np.float16)
    # layer-2: W1_dev[p, o, k, m] = A1[128o+m, 128k+p] (f stores channel
    # 128k+p at partition p of column-block k).
    A4 = A1.reshape(OC2, 128, KC2, 128)              # [o, m, k, p]
    w1 = np.ascontiguousarray(
        A4.transpose(3, 0, 2, 1).reshape(128, OC2 * KC2 * 128)
    ).astype(np.float16)
    w2 = np.ascontiguousarray(g["W2"].reshape(OC2, 128).T).astype(np.float16)
    # bias pack [128, 10] fp32: bso(4) | bc(4) | b1(2)
    bias = np.concatenate([
        a_so.reshape(OC1, 128).T, a_c.reshape(OC1, 128).T,
        a1.reshape(OC2, 128).T], axis=1)
    bias = np.ascontiguousarray(bias).astype(np.float32)
    thresh = float(-HW * g["b2"][0])
    return wso, wc, w1, w2, bias, thresh


def build_bass(thresh, repeat=1, loop=1):
    nc = bacc.Bacc("TRN2", target_bir_lowering=False, debug=False)

    xso_d = nc.dram_tensor("x_so", [BPC, C, NHW], F16, kind="ExternalInput").ap()
    xc_d = nc.dram_tensor("x_c", [BPC, C, NHW], F16, kind="ExternalInput").ap()
    # combined streams: k-chunks of (all weight o-blocks | x[b0]) per
    # branch - each branch's whole working set in consumption order.
    cso_d = nc.dram_tensor("comb_so", [128, KC1 * RC], F16, kind="ExternalInput").ap()
    cc_d = nc.dram_tensor("comb_c", [128, KC1 * RC], F16, kind="ExternalInput").ap()
    w1_d = nc.dram_tensor("w1", [128, OC2 * KC2 * 128], F16, kind="ExternalInput").ap()
    w2_d = nc.dram_tensor("w2", [128, OC2], F16, kind="ExternalInput").ap()
    bias_d = nc.dram_tensor("bias", [128, 2 * OC1 + OC2], F32, kind="ExternalInput").ap()
    out_d = nc.dram_tensor("out", [BPC * NN], F32, kind="ExternalOutput").ap()

    with tile.TileContext(nc) as tc:
        with (
            tc.tile_pool(name="wp", bufs=1) as wp,
            tc.tile_pool(name="xp", bufs=2) as xp,
            tc.tile_pool(name="fp", bufs=2) as fp,
            tc.tile_pool(name="hp", bufs=2) as hp,
            tc.tile_pool(name="ap", bufs=1) as ac,
            tc.tile_pool(name="ps1", bufs=4, space="PSUM") as ps1,
            tc.tile_pool(name="ps2", bufs=2, space="PSUM") as ps2,
            tc.tile_pool(name="ps3", bufs=2, space="PSUM") as ps3,
        ):
            # ---- tiny tensors on the gpsimd (SWDGE) ring ----
            # memset first: it gates the PE pre-warm matmuls.
            dummy_sb = wp.tile([128, 128], F16)
            nc.gpsimd.memset(dummy_sb[:], 0)
            bias_sb = wp.tile([128, 2 * OC1 + OC2], F32)
            nc.gpsimd.dma_start(bias_sb[:], bias_d[:])
            w2_sb = wp.tile([128, OC2], F16)
            nc.gpsimd.dma_start(w2_sb[:], w2_d[:])

            # combined-stream tiles stay resident: steady batches read
            # layer-1 weights from them.
            cso_sb = wp.tile([128, KC1 * RC], F16)
            cc_sb = wp.tile([128, KC1 * RC], F16)
            w1_sb = wp.tile([128, OC2 * KC2 * 128], F16)

            # ---- PE pre-warm: HAM flips to 2.4GHz after ~3.4us of
            # activity; burn the DMA lead-in on dummy matmuls so the
            # real stream runs warm from the first real k-chunk.
            wps = ps1.tile([128, MT], F32, tag="ps1")
            for i in range(N_DUMMY):
                nc.tensor.matmul(wps[:, 0:128], lhsT=dummy_sb[:],
                                 rhs=dummy_sb[:], start=True, stop=True)

            out_sb = ac.tile([1, BPC * NN], F32)
            bits_sb = ac.tile([1, BPC * NN], F32)

            import contextlib
            loop_cm = tc.For_i(0, loop, 1) if loop > 1 else contextlib.nullcontext()
            with loop_cm:
                _body(nc, tc, repeat, xso_d, xc_d, (cso_d, cc_d, w1_d),
                      out_d, (cso_sb, cc_sb), w1_sb, w2_sb,
                      bias_sb, out_sb, bits_sb, xp, fp, hp,
                      ps1, ps2, ps3, thresh)

    nc.compile()
    return nc


def _body(nc, tc, repeat, xso_d, xc_d, comb_dram, out_d,
          comb_sb, w1_sb, w2_sb, bias_sb,
          out_sb, bits_sb, xp, fp, hp, ps1, ps2, ps3, thresh):
    cso_d, cc_d, w1_d = comb_dram
    comb_t = [sb.rearrange("p (k c) -> p k c", k=KC1) for sb in comb_sb]
    comb_v = [dd.rearrange("p (k c) -> p k c", k=KC1) for dd in (cso_d, cc_d)]

    def x_sub(ring, x_sb, x_d, b, lo, hi):
        """DMA k-chunks [lo, hi) of x[b]."""
        x_t = x_sb.rearrange("p (j m) -> p j m", j=KC1)
        x_v = x_d[b].rearrange("(p j) m -> p j m", p=128)
        ring.dma_start(x_t[:, lo:hi, :], x_v[:, lo:hi, :])

    def l1_lhsT(br, o):
        return lambda k: comb_t[br][:, k, o * 128:(o + 1) * 128]

    def l1_act(ps, f_sb, br, o, m):
        col = (br * OC1 + o) * NHW + m * MT
        boff = br * OC1 + o
        nc.scalar.activation(
            f_sb[:, col:col + MT], ps[:],
            mybir.ActivationFunctionType.Prelu,
            bias=bias_sb[:, boff:boff + 1], scale=1.0, alpha=SLOPE)

    def l3(b, h_sb, m3, last=False):
        """layer 3 (y = W2 @ h) for one m-half + 49-group mean-reduce."""
        ps = ps3.tile([1, MT], F32, tag="ps3")
        q_order = (1, 0) if (last and m3 == 1) else (0, 1)
        for qi, q in enumerate(q_order):
            nc.tensor.matmul(
                ps[:], lhsT=w2_sb[:, q:q + 1],
                rhs=h_sb[:, q * NHW + m3 * MT:q * NHW + m3 * MT + MT],
                start=(qi == 0), stop=(qi == OC2 - 1))
        off = b * NN + m3 * (MT // HW)
        nc.vector.reduce_sum(
            out_sb[0:1, off:off + MT // HW],
            ps.rearrange("p (g x) -> p g x", x=HW),
            axis=mybir.AxisListType.X)

    def finish(b):
        """threshold + store for batch b:
        sigmoid(mean) > 0.5  <=>  sum > -49*b2"""
        nc.vector.tensor_scalar(
            bits_sb[0:1, b * NN:(b + 1) * NN],
            out_sb[0:1, b * NN:(b + 1) * NN], float(thresh), None,
            mybir.AluOpType.is_gt)
        # b<last: gpsimd (SWDGE) ring - a sync-ring store would block
        # later x-transfer triggers queued behind it. Last batch: sync
        # ring (HWDGE completes ~0.6us faster, nothing queues after).
        ring_out = nc.sync if b == BPC - 1 else nc.gpsimd
        ring_out.dma_start(out_d[b * NN:(b + 1) * NN],
                           bits_sb[0:1, b * NN:(b + 1) * NN])

    pending = None

    def l1_8g(f_sb, br, rhs_of):
        """whole branch with 8 PSUM groups (o,m) open and k-chunks
        interleaved: each arriving chunk (x 147KB + w 128KB) feeds 8
        matmuls (1us), so PE demand matches even the cold DMA rate and
        per-chunk waits stay far below the HAM idle threshold."""
        lhs = [l1_lhsT(br, o) for o in range(OC1)]
        pools = (ps1, ps1, ps2, ps3)
        tags = ("ps1", "ps1", "ps2", "ps3")
        tiles = {}
        for o in range(OC1):
            for m in range(2):
                g_ps = pools[o].tile([128, MT], F32, tag=tags[o])
                tiles[(o, m)] = g_ps
        for k in range(KC1):
            for m in range(2):
                for o in range(OC1):
                    nc.tensor.matmul(
                        tiles[(o, m)][:], lhsT=lhs[o](k), rhs=rhs_of(k, m),
                        start=(k == 0), stop=(k == KC1 - 1))
        for o in range(OC1):
            for m in range(2):
                l1_act(tiles[(o, m)], f_sb, br, o, m)

    for _rep in range(repeat):
        for b in range(BPC):
            first = _rep == 0 and b == 0
            if first:
                # ---- ramp trigger schedule ----
                # the so-stream owns BOTH rings in alternating, growing
                # transfers (flat-column ranges; the first covers just
                # chunk 0's weights + m0 x so the first matmuls start
                # ~1.5us earlier); the c-stream follows, done by ~28us.
                # The first 8 triggers use the 8 HWDGE semaphores
                # exactly once, so no critical trigger ever waits.
                rings = (nc.sync, nc.scalar)
                so_cuts = (0, XO + MT, 2 * RC, 3 * RC, 4 * RC, 5 * RC,
                           6 * RC, 8 * RC, 10 * RC, 12 * RC, 14 * RC,
                           KC1 * RC)
                for i, (lo, hi) in enumerate(
                        zip(so_cuts, so_cuts[1:])):
                    rings[i % 2].dma_start(comb_sb[0][:, lo:hi],
                                           cso_d[:, lo:hi])
                for i in range(8):
                    rings[(i + 1) % 2].dma_start(
                        comb_t[1][:, 2 * i:2 * i + 2, :],
                        comb_v[1][:, 2 * i:2 * i + 2, :])
                with tc.tile_wait_until(0.018):
                    nc.scalar.dma_start(w1_sb[:], w1_d[:])
            else:
                # steady-state x loads: one sync-ring trigger per
                # tensor, scheduler-delayed so they queue behind the
                # batch-0 streams; data still lands ~30us before use.
                delay = 0.020 + 0.004 * (_rep * BPC + b)
                with tc.tile_wait_until(delay):
                    xso_sb = xp.tile([128, KC1 * NHW], F16, tag="xso")
                    x_sub(nc.sync, xso_sb, xso_d, b, 0, KC1)
                with tc.tile_wait_until(delay + 0.002):
                    xc_sb = xp.tile([128, KC1 * NHW], F16, tag="xc")
                    x_sub(nc.sync, xc_sb, xc_d, b, 0, KC1)

            # ---- layer 1: f = lrelu(A @ x + a), fp16 out ----
            f_sb = fp.tile([128, 2 * OC1 * NHW], F16, tag="f")
            if first:
                for br in range(2):
                    l1_8g(f_sb, br,
                          lambda k, m, br=br: comb_t[br][:, k,
                                                         XO + m * MT:
                                                         XO + (m + 1) * MT])
            else:
                for br, x_sb in enumerate((xso_sb, xc_sb)):
                    rhs = lambda k, m, x_sb=x_sb: x_sb[
                        :, k * NHW + m * MT:k * NHW + m * MT + MT]
                    for o in range(OC1):
                        for m in range(2):
                            ps = ps1.tile([128, MT], F32, tag="ps1")
                            for k in range(KC1):
                                nc.tensor.matmul(
                                    ps[:], lhsT=l1_lhsT(br, o)(k),
                                    rhs=rhs(k, m),
                                    start=(k == 0), stop=(k == KC1 - 1))
                            l1_act(ps, f_sb, br, o, m)
                            if br == 0 and o == 0 and m == 1 and pending:
                                # previous batch's deferred L3-m1 +
                                # threshold + store: its L2 acts are
                                # ~2us old now, so nothing stalls.
                                l3(*pending, 1)
                                finish(pending[0])
                                pending = None

            # ---- layer 2: h = lrelu(A1 @ f + a1); layer 3 + mean-reduce:
            # y = W2 @ h ; sum 49-groups. L3-m0 runs inline (its h acts
            # are old); L3-m1 + threshold + store are DEFERRED into the
            # next batch's L1 stream so the last L2 act's latency hides
            # under matmuls. Last batch: L3-m runs right after L2-m with
            # the o/q-order trick so only one act is on the tail.
            h_sb = hp.tile([128, OC2 * NHW], F16, tag="h")
            last = b == BPC - 1
            for m in range(2):
                # last batch, m1: o1 first so act(o1) runs under o0's
                # matmuls; L3 then accumulates q1 before q0 (exact - a
                # two-term fp32 add commutes), leaving only the o0 act
                # on the tail's critical path.
                o_order = (1, 0) if (last and m == 1) else (0, 1)
                for o in o_order:
                    ps = ps2.tile([128, MT], F32, tag="ps2")
                    for k in range(KC2):
                        nc.tensor.matmul(
                            ps[:],
                            lhsT=w1_sb[:, (o * KC2 + k) * 128:
                                       (o * KC2 + k) * 128 + 128],
                            rhs=f_sb[:, k * NHW + m * MT:
                                     k * NHW + m * MT + MT],
                            start=(k == 0), stop=(k == KC2 - 1))
                    col = o * NHW + m * MT
                    nc.scalar.activation(
                        h_sb[:, col:col + MT], ps[:],
                        mybir.ActivationFunctionType.Prelu,
                        bias=bias_sb[:, 2 * OC1 + o:2 * OC1 + o + 1],
                        scale=1.0, alpha=SLOPE)
                for m3 in ([m] if last else ([0] if m == 1 else [])):
                    l3(b, h_sb, m3, last)
            if last:
                finish(b)
            else:
                pending = (b, h_sb)


_CACHE = {}


def _get_nc(thresh, repeat=1, loop=1):
    key = (round(thresh, 9), repeat, loop)
    if key not in _CACHE:
        _CACHE[key] = build_bass(thresh, repeat, loop)
    return _CACHE[key]


def _prepare(inputs):
    """Fold params, cast x to fp16, build per-core input maps + nc."""
    wso, wc, w1, w2, bias, thresh = _fold_params(inputs)
    xso = np.asarray(inputs["x_so"], dtype=np.float32).reshape(
        B, C, NHW).astype(np.float16)
    xc = np.asarray(inputs["x_c"], dtype=np.float32).reshape(
        B, C, NHW).astype(np.float16)
    in_maps = []
    for i in range(N_CORES):
        # combined streams: per k-chunk, all weight o-blocks (512 cols)
        # then x[b0] (588 cols) - b0's data in consumption order.
        cso = np.concatenate(
            [wso, xso[i * BPC].reshape(128, KC1, NHW)], axis=2)
        cc = np.concatenate(
            [wc, xc[i * BPC].reshape(128, KC1, NHW)], axis=2)
        in_maps.append({
            "x_so": xso[i * BPC:(i + 1) * BPC],
            "x_c": xc[i * BPC:(i + 1) * BPC],
            "comb_so": np.ascontiguousarray(cso.reshape(128, KC1 * RC)),
            "comb_c": np.ascontiguousarray(cc.reshape(128, KC1 * RC)),
            "w1": w1, "w2": w2, "bias": bias,
        })
    return _get_nc(thresh), in_maps


def kernel(**inputs):
    nc, in_maps = _prepare(inputs)
    res = run_bass_kernel_spmd(nc, in_maps, list(range(N_CORES)))
    out = np.concatenate([res.results[i]["out"].reshape(BPC, NN)
                          for i in range(N_CORES)], axis=0)
    return np.ascontiguousarray(out.reshape(B, NN, 1).astype(np.float32))


# revision 44
# speedup vs baseline: 1.0576x; 1.0012x over previous
"""Trainium2 Bass kernel for nn_ContextGatingSigmoidClassifier.

Math (eval mode):
  f_so = lrelu(W_so @ bn_so(x_so) + b_so)        x: [B,2048,N,H,W]
  f_c  = lrelu(W_c  @ bn_c(x_c)  + b_c)
  f    = concat -> bn1 -> W1 -> bn2 -> lrelu -> W2 -> mean(H,W) -> sigmoid > 0.5

All BatchNorms are eval-mode affine maps, so they fold into the adjacent
linear layers (done host-side in fp64). Final threshold:
  sigmoid(mean) > 0.5  <=>  sum_hw(W2 @ h) > -49*b2.

Device mapping: data-parallel over batch (4 per core, 8 cores), weights
replicated, x cast fp32->fp16 host-side. The kernel is tensor-engine
bound (1152 matmuls of N=294 at 125ns = 144us); the schedule exists to
keep the PE stream dense from the first possible cycle. Measured HW
facts that shape it:
  - ~7.1us of fixed BSP preamble before any kernel instruction.
  - Both HWDGE rings share ~420 GB/s; each transfer costs ~0.6-1us of
    fixed overhead (trigger + descgen), and the first ~4us of DMA
    activity runs at roughly half rate (cold clock).
  - A PE idle gap of more than ~2-3us re-gates the PE clock to 1.2GHz
    for 3.4us+ (HAM) - stalls snowball. So batch 0 runs layer 1 with
    an 8-PSUM-group schedule (4 o-chunks x 2 m-tiles at once): each
    k-chunk of data (x 147KB + weights 128KB) feeds 8 matmuls (1us),
    so PE demand (~275 GB/s) roughly matches even the cold DMA rate
    and per-chunk waits stay well under the HAM threshold.
  - Each of batch 0's branches gets its whole working set (4 weight
    o-blocks + x, 4.4MB) packed host-side into ONE k-ordered stream,
    split across both rings in alternating, geometrically growing
    transfers. The first 8 transfers use the 8 HWDGE semaphores at
    most once, so no critical trigger ever waits (the Tile framework
    recycles semaphores round-robin and a recycled trigger blocks on
    the previous user's completion); later triggers' recycled waits
    land on long-completed transfers.
  - w1 and the b1-b3 x loads carry manual scheduler delays
    (tile_wait_until) so the scheduler cannot hoist these dependency-
    free triggers in front of the critical streams.
  - PE pre-warm: dummy matmuls fill the PE queue from the preamble end
    (~7.4us) to the first k-chunk arrival (~10us).
  - Steady batches (b1-b3) use the o-major loop (2-deep PSUM slack vs
    activation latency), reading layer-1 weights from the resident
    stream tiles; all input DMA lands by ~70us of a ~165us kernel.
Per batch element: x[b] is [2048, 588] (channels on SBUF partitions in
K-chunks of 128, positions on the free dim, 2 m-tiles of 294 = one PSUM
bank each). Channel->partition mapping is interleaved (partition p holds
channels 16p..16p+15) so every DMA descriptor is one contiguous
per-partition run; weights are permuted host-side to match.
"""

import numpy as np

import concourse.bass as bass  # noqa: F401
import concourse.tile as tile
from concourse import bacc, mybir
from concourse.bass_utils import run_bass_kernel_spmd

F16 = mybir.dt.float16
F32 = mybir.dt.float32

B, C, NN, HW = 32, 2048, 12, 49
NHW = NN * HW            # 588
N_CORES = 8
BPC = B // N_CORES       # 4 batch elements per core
MT = NHW // 2            # 294 columns = one PSUM bank of fp32
KC1 = C // 128           # 16 K-chunks, layer 1
OC1 = 512 // 128         # 4 output chunks, layer 1 (per branch)
KC2 = 1024 // 128        # 8 K-chunks, layer 2
OC2 = 256 // 128         # 2 output chunks, layer 2
EPS = 1e-5
SLOPE = 0.2
N_DUMMY = 14             # PE pre-warm matmuls (294-col, ~245ns each at
                         # 1.2GHz); sized to bridge the PE from the
                         # preamble end (~7.4us) to k0 arrival (~10.8us)
                         # with NO gap: the HAM clock needs ~3.4us of
                         # continuous DENSE activity, and an idle gap
                         # resets it (costing ~4us of half-clock on the
                         # real stream).
XO = OC1 * 128           # x column offset inside a combined chunk
RC = XO + NHW            # 1100 = combined chunk: weights (4 o) + x cols


def _quad(A):
    """[512, 2048] -> [128, KC1, 512] k-major weight layout:
    Wq[p, k, o, m] = A[128o+m, 16p+k] (chunk k holds channel 16p+k at
    partition p, matching the x DMA layout)."""
    A4 = A.reshape(OC1, 128, 128, KC1)               # [o, m, p, k]
    return np.ascontiguousarray(
        A4.transpose(2, 3, 0, 1).reshape(128, KC1, OC1 * 128))


def _fold_params(d):
    """Fold BNs into linears, in fp64. Returns device-layout arrays."""
    g = {k: np.asarray(v, dtype=np.float64) for k, v in d.items()}

    def bn_st(p):
        s = g[f"{p}_g"] / np.sqrt(g[f"{p}_v"] + EPS)
        t = g[f"{p}_b"] - g[f"{p}_m"] * s
        return s, t

    s_so, t_so = bn_st("bn_so")
    s_c, t_c = bn_st("bn_c")
    s1, t1 = bn_st("bn1")
    s2, t2 = bn_st("bn2")

    A_so = g["W_so"] * s_so[None, :]                 # [512, 2048]
    a_so = g["W_so"] @ t_so + g["b_so"]              # [512]
    A_c = g["W_c"] * s_c[None, :]
    a_c = g["W_c"] @ t_c + g["b_c"]
    A1 = s2[:, None] * (g["W1"] * s1[None, :])       # [256, 1024]
    a1 = s2 * (g["W1"] @ t1 + g["b1"]) + t2          # [256]

    wso = _quad(A_so).astype(np.float16)
    wc = _quad(A_c).astype(np.float16)
    # layer-2: W1_dev[p, o, k, m] = A1[128o+m, 128k+p] (f stores channel
    # 128k+p at partition p of column-block k).
    A4 = A1.reshape(OC2, 128, KC2, 128)              # [o, m, k, p]
    w1 = np.ascontiguousarray(
        A4.transpose(3, 0, 2, 1).reshape(128, OC2 * KC2 * 128)
    ).astype(np.float16)
    w2 = np.ascontiguousarray(g["W2"].reshape(OC2, 128).T).astype(np.float16)
    # bias pack [128, 10] fp32: bso(4) | bc(4) | b1(2)
    bias = np.concatenate([
        a_so.reshape(OC1, 128).T, a_c.reshape(OC1, 128).T,
        a1.reshape(OC2, 128).T], axis=1)
    bias = np.ascontiguousarray(bias).astype(np.float32)
    thresh = float(-HW * g["b2"][0])
    return wso, wc, w1, w2, bias, thresh


def build_bass(thresh, repeat=1, loop=1):
    nc = bacc.Bacc("TRN2", target_bir_lowering=False, debug=False)

    xso_d = nc.dram_tensor("x_so", [BPC, C, NHW], F16, kind="ExternalInput").ap()
    xc_d = nc.dram_tensor("x_c", [BPC, C, NHW], F16, kind="ExternalInput").ap()
    # combined streams: k-chunks of (all weight o-blocks | x[b0]) per
    # branch - each branch's whole working set in consumption order.
    cso_d = nc.dram_tensor("comb_so", [128, KC1 * RC], F16, kind="ExternalInput").ap()
    cc_d = nc.dram_tensor("comb_c", [128, KC1 * RC], F16, kind="ExternalInput").ap()
    w1_d = nc.dram_tensor("w1", [128, OC2 * KC2 * 128], F16, kind="ExternalInput").ap()
    w2_d = nc.dram_tensor("w2", [128, OC2], F16, kind="ExternalInput").ap()
    bias_d = nc.dram_tensor("bias", [128, 2 * OC1 + OC2], F32, kind="ExternalInput").ap()
    out_d = nc.dram_tensor("out", [BPC * NN], F32, kind="ExternalOutput").ap()

    with tile.TileContext(nc) as tc:
        with (
            tc.tile_pool(name="wp", bufs=1) as wp,
            tc.tile_pool(name="xp", bufs=2) as xp,
            tc.tile_pool(name="fp", bufs=2) as fp,
            tc.tile_pool(name="hp", bufs=2) as hp,
            tc.tile_pool(name="ap", bufs=1) as ac,
            tc.tile_pool(name="ps1", bufs=4, space="PSUM") as ps1,
            tc.tile_pool(name="ps2", bufs=2, space="PSUM") as ps2,
            tc.tile_pool(name="ps3", bufs=2, space="PSUM") as ps3,
        ):
            # ---- tiny tensors on the gpsimd (SWDGE) ring ----
            # memset first: it gates the PE pre-warm matmuls.
            dummy_sb = wp.tile([128, MT], F16)
            nc.gpsimd.memset(dummy_sb[:], 0)
            bias_sb = wp.tile([128, 2 * OC1 + OC2], F32)
            nc.gpsimd.dma_start(bias_sb[:], bias_d[:])
            w2_sb = wp.tile([128, OC2], F16)
            nc.gpsimd.dma_start(w2_sb[:], w2_d[:])

            # combined-stream tiles stay resident: steady batches read
            # layer-1 weights from them.
            cso_sb = wp.tile([128, KC1 * RC], F16)
            cc_sb = wp.tile([128, KC1 * RC], F16)
            w1_sb = wp.tile([128, OC2 * KC2 * 128], F16)

            # ---- PE pre-warm: HAM flips to 2.4GHz after ~3.4us of
            # activity; burn the DMA lead-in on dummy matmuls so the
            # real stream runs warm from the first real k-chunk.
            wps = ps1.tile([128, MT], F32, tag="ps1")
            for i in range(N_DUMMY):
                nc.tensor.matmul(wps[:], lhsT=dummy_sb[:, 0:128],
                                 rhs=dummy_sb[:], start=True, stop=True)

            out_sb = ac.tile([1, BPC * NN], F32)
            bits_sb = ac.tile([1, BPC * NN], F32)

            import contextlib
            loop_cm = tc.For_i(0, loop, 1) if loop > 1 else contextlib.nullcontext()
            with loop_cm:
                _body(nc, tc, repeat, xso_d, xc_d, (cso_d, cc_d, w1_d),
                      out_d, (cso_sb, cc_sb), w1_sb, w2_sb,
                      bias_sb, out_sb, bits_sb, xp, fp, hp,
                      ps1, ps2, ps3, thresh)

    nc.compile()
    return nc


def _body(nc, tc, repeat, xso_d, xc_d, comb_dram, out_d,
          comb_sb, w1_sb, w2_sb, bias_sb,
          out_sb, bits_sb, xp, fp, hp, ps1, ps2, ps3, thresh):
    cso_d, cc_d, w1_d = comb_dram
    comb_t = [sb.rearrange("p (k c) -> p k c", k=KC1) for sb in comb_sb]
    comb_v = [dd.rearrange("p (k c) -> p k c", k=KC1) for dd in (cso_d, cc_d)]

    def x_sub(ring, x_sb, x_d, b, lo, hi):
        """DMA k-chunks [lo, hi) of x[b]."""
        x_t = x_sb.rearrange("p (j m) -> p j m", j=KC1)
        x_v = x_d[b].rearrange("(p j) m -> p j m", p=128)
        ring.dma_start(x_t[:, lo:hi, :], x_v[:, lo:hi, :])

    def l1_lhsT(br, o):
        return lambda k: comb_t[br][:, k, o * 128:(o + 1) * 128]

    def l1_act(ps, f_sb, br, o, m):
        col = (br * OC1 + o) * NHW + m * MT
        boff = br * OC1 + o
        nc.scalar.activation(
            f_sb[:, col:col + MT], ps[:],
            mybir.ActivationFunctionType.Prelu,
            bias=bias_sb[:, boff:boff + 1], scale=1.0, alpha=SLOPE)

    def l3(b, h_sb, m3, last=False):
        """layer 3 (y = W2 @ h) for one m-half + 49-group mean-reduce."""
        ps = ps3.tile([1, MT], F32, tag="ps3")
        q_order = (1, 0) if (last and m3 == 1) else (0, 1)
        for qi, q in enumerate(q_order):
            nc.tensor.matmul(
                ps[:], lhsT=w2_sb[:, q:q + 1],
                rhs=h_sb[:, q * NHW + m3 * MT:q * NHW + m3 * MT + MT],
                start=(qi == 0), stop=(qi == OC2 - 1))
        off = b * NN + m3 * (MT // HW)
        nc.vector.reduce_sum(
            out_sb[0:1, off:off + MT // HW],
            ps.rearrange("p (g x) -> p g x", x=HW),
            axis=mybir.AxisListType.X)

    def finish(b):
        """threshold + store for batch b:
        sigmoid(mean) > 0.5  <=>  sum > -49*b2"""
        nc.vector.tensor_scalar(
            bits_sb[0:1, b * NN:(b + 1) * NN],
            out_sb[0:1, b * NN:(b + 1) * NN], float(thresh), None,
            mybir.AluOpType.is_gt)
        # b<last: gpsimd (SWDGE) ring - a sync-ring store would block
        # later x-transfer triggers queued behind it. Last batch: sync
        # ring (HWDGE completes ~0.6us faster, nothing queues after).
        ring_out = nc.sync if b == BPC - 1 else nc.gpsimd
        ring_out.dma_start(out_d[b * NN:(b + 1) * NN],
                           bits_sb[0:1, b * NN:(b + 1) * NN])

    pending = None

    def l1_8g(f_sb, br, rhs_of):
        """whole branch with 8 PSUM groups (o,m) open and k-chunks
        interleaved: each arriving chunk (x 147KB + w 128KB) feeds 8
        matmuls (1us), so PE demand matches even the cold DMA rate and
        per-chunk waits stay far below the HAM idle threshold."""
        lhs = [l1_lhsT(br, o) for o in range(OC1)]
        pools = (ps1, ps1, ps2, ps3)
        tags = ("ps1", "ps1", "ps2", "ps3")
        tiles = {}
        for o in range(OC1):
            for m in range(2):
                g_ps = pools[o].tile([128, MT], F32, tag=tags[o])
                tiles[(o, m)] = g_ps
        for k in range(KC1):
            for m in range(2):
                for o in range(OC1):
                    nc.tensor.matmul(
                        tiles[(o, m)][:], lhsT=lhs[o](k), rhs=rhs_of(k, m),
                        start=(k == 0), stop=(k == KC1 - 1))
        for o in range(OC1):
            for m in range(2):
                l1_act(tiles[(o, m)], f_sb, br, o, m)

    for _rep in range(repeat):
        for b in range(BPC):
            first = _rep == 0 and b == 0
            if first:
                # ---- ramp trigger schedule ----
                # the so-stream owns BOTH rings in alternating, growing
                # transfers (flat-column ranges; the first covers just
                # chunk 0's weights + m0 x so the first matmuls start
                # ~1.5us earlier); the c-stream follows, done by ~28us.
                # The first 8 triggers use the 8 HWDGE semaphores
                # exactly once, so no critical trigger ever waits.
                rings = (nc.sync, nc.scalar)
                so_cuts = (0, XO + MT, 2 * RC, 3 * RC, 4 * RC, 5 * RC,
                           6 * RC, 8 * RC, 10 * RC, 12 * RC, 14 * RC,
                           KC1 * RC)
                for i, (lo, hi) in enumerate(
                        zip(so_cuts, so_cuts[1:])):
                    rings[i % 2].dma_start(comb_sb[0][:, lo:hi],
                                           cso_d[:, lo:hi])
                for i in range(8):
                    rings[(i + 1) % 2].dma_start(
                        comb_t[1][:, 2 * i:2 * i + 2, :],
                        comb_v[1][:, 2 * i:2 * i + 2, :])
                with tc.tile_wait_until(0.018):
                    nc.scalar.dma_start(w1_sb[:], w1_d[:])
            else:
                # steady-state x loads: one sync-ring trigger per
                # tensor, scheduler-delayed so they queue behind the
                # batch-0 streams; data still lands ~30us before use.
                delay = 0.020 + 0.004 * (_rep * BPC + b)
                with tc.tile_wait_until(delay):
                    xso_sb = xp.tile([128, KC1 * NHW], F16, tag="xso")
                    x_sub(nc.sync, xso_sb, xso_d, b, 0, KC1)
                with tc.tile_wait_until(delay + 0.002):
                    xc_sb = xp.tile([128, KC1 * NHW], F16, tag="xc")
                    x_sub(nc.sync, xc_sb, xc_d, b, 0, KC1)

            # ---- layer 1: f = lrelu(A @ x + a), fp16 out ----
            f_sb = fp.tile([128, 2 * OC1 * NHW], F16, tag="f")
            if first:
                for br in range(2):
                    l1_8g(f_sb, br,
                          lambda k, m, br=br: comb_t[br][:, k,
                                                         XO + m * MT:
                                                         XO + (m + 1) * MT])
            else:
                for br, x_sb in enumerate((xso_sb, xc_sb)):
                    rhs = lambda k, m, x_sb=x_sb: x_sb[
                        :, k * NHW + m * MT:k * NHW + m * MT + MT]
                    for o in range(OC1):
                        for m in range(2):
                            ps = ps1.tile([128, MT], F32, tag="ps1")
                            for k in range(KC1):
                                nc.tensor.matmul(
                                    ps[:], lhsT=l1_lhsT(br, o)(k),
                                    rhs=rhs(k, m),
                                    start=(k == 0), stop=(k == KC1 - 1))
                            l1_act(ps, f_sb, br, o, m)
                            if br == 0 and o == 0 and m == 1 and pending:
                                # previous batch's deferred L3-m1 +
                                # threshold + store: its L2 acts are
                                # ~2us old now, so nothing stalls.
                                l3(*pending, 1)
                                finish(pending[0])
                                pending = None

            # ---- layer 2: h = lrelu(A1 @ f + a1); layer 3 + mean-reduce:
            # y = W2 @ h ; sum 49-groups. L3-m0 runs inline (its h acts
            # are old); L3-m1 + threshold + store are DEFERRED into the
            # next batch's L1 stream so the last L2 act's latency hides
            # under matmuls. Last batch: L3-m runs right after L2-m with
            # the o/q-order trick so only one act is on the tail.
            h_sb = hp.tile([128, OC2 * NHW], F16, tag="h")
            last = b == BPC - 1
            for m in range(2):
                # last batch, m1: o1 first so act(o1) runs under o0's
                # matmuls; L3 then accumulates q1 before q0 (exact - a
                # two-term fp32 add commutes), leaving only the o0 act
                # on the tail's critical path.
                o_order = (1, 0) if (last and m == 1) else (0, 1)
                for o in o_order:
                    ps = ps2.tile([128, MT], F32, tag="ps2")
                    for k in range(KC2):
                        nc.tensor.matmul(
                            ps[:],
                            lhsT=w1_sb[:, (o * KC2 + k) * 128:
                                       (o * KC2 + k) * 128 + 128],
                            rhs=f_sb[:, k * NHW + m * MT:
                                     k * NHW + m * MT + MT],
                            start=(k == 0), stop=(k == KC2 - 1))
                    col = o * NHW + m * MT
                    nc.scalar.activation(
                        h_sb[:, col:col + MT], ps[:],
                        mybir.ActivationFunctionType.Prelu,
                        bias=bias_sb[:, 2 * OC1 + o:2 * OC1 + o + 1],
                        scale=1.0, alpha=SLOPE)
                for m3 in ([m] if last else ([0] if m == 1 else [])):
                    l3(b, h_sb, m3, last)
            if last:
                finish(b)
            else:
                pending = (b, h_sb)


_CACHE = {}


def _get_nc(thresh, repeat=1, loop=1):
    key = (round(thresh, 9), repeat, loop)
    if key not in _CACHE:
        _CACHE[key] = build_bass(thresh, repeat, loop)
    return _CACHE[key]


def _prepare(inputs):
    """Fold params, cast x to fp16, build per-core input maps + nc."""
    wso, wc, w1, w2, bias, thresh = _fold_params(inputs)
    xso = np.asarray(inputs["x_so"], dtype=np.float32).reshape(
        B, C, NHW).astype(np.float16)
    xc = np.asarray(inputs["x_c"], dtype=np.float32).reshape(
        B, C, NHW).astype(np.float16)
    in_maps = []
    for i in range(N_CORES):
        # combined streams: per k-chunk, all weight o-blocks (512 cols)
        # then x[b0] (588 cols) - b0's data in consumption order.
        cso = np.concatenate(
            [wso, xso[i * BPC].reshape(128, KC1, NHW)], axis=2)
        cc = np.concatenate(
            [wc, xc[i * BPC].reshape(128, KC1, NHW)], axis=2)
        in_maps.append({
            "x_so": xso[i * BPC:(i + 1) * BPC],
            "x_c": xc[i * BPC:(i + 1) * BPC],
            "comb_so": np.ascontiguousarray(cso.reshape(128, KC1 * RC)),
            "comb_c": np.ascontiguousarray(cc.reshape(128, KC1 * RC)),
            "w1": w1, "w2": w2, "bias": bias,
        })
    return _get_nc(thresh), in_maps


def kernel(**inputs):
    nc, in_maps = _prepare(inputs)
    res = run_bass_kernel_spmd(nc, in_maps, list(range(N_CORES)))
    out = np.concatenate([res.results[i]["out"].reshape(BPC, NN)
                          for i in range(N_CORES)], axis=0)
    return np.ascontiguousarray(out.reshape(B, NN, 1).astype(np.float32))
